# revision 12
# baseline (speedup 1.0000x reference)
"""ETSFormer forward pass on 8 Trainium2 NeuronCores (Bass/Tile).

Data-parallel over batch: 32 samples -> 8 cores x 4 samples, weights
replicated, no collectives. The reference's FFT machinery is computed
exactly without FFTs:
  - freq_attention: dense DFT matmuls + hardware top-8 (vector.max) mask
  - mhesa / level exponential smoothing: the reference FFT cross-correlation
    is exactly a first-order EMA -> hardware prefix scan (tensor_tensor_scan)
  - fourier_extrapolate: exact slice (Dirichlet kernel identity)

Precision: the top-4 frequency mask is extremely sensitive (2e-4 relative
amp noise -> 2.6e-2 output error), so every GEMM feeding a ranking (conv,
rfft both layers, irfft/mhesa/FF of layer 0) runs in fp32; post-ranking
paths (layer-1 irfft/mhesa via lgT, level, damp, output head) run fp32r.
"""
import numpy as np
from contextlib import ExitStack

import concourse.bass as bass
import concourse.bacc as bacc
import concourse.tile as tile
from concourse import mybir
from concourse.bass_utils import run_bass_kernel_spmd

F32 = mybir.dt.float32
F32R = mybir.dt.float32r
AF = mybir.ActivationFunctionType
ALU = mybir.AluOpType

N = 1024
D = 512
TF = 7
HEADS = 8
DH = D // HEADS
L = 2
S = 4
NCORES = 8
HOR = 96
FD = 2048
NT = N // 128   # 8
ND = D // 128   # 4
NM = FD // 128  # 16

_CACHE = {}
OMA_BCAST = True


def _dft_consts():
    if "dft" not in _CACHE:
        t = np.arange(N)
        f = np.arange(513)
        ang = 2.0 * np.pi * np.outer(t, f) / N
        cos = np.cos(ang)
        sin = np.sin(ang)
        dft = np.zeros((N, 1024), np.float64)
        dft[:, 0:512] = cos[:, 0:512]
        dft[:, 512] = cos[:, 512]
        dft[:, 513:1024] = sin[:, 1:512]
        c = np.full(513, 2.0)
        c[0] = 1.0
        c[512] = 1.0
        ib = np.zeros((1024, N), np.float64)
        ib[0:512, :] = (c[0:512, None] / N) * cos[:, 0:512].T
        ib[512, :] = (1.0 / N) * cos[:, 512]
        ib[513:1024, :] = (2.0 / N) * sin[:, 1:512].T
        _CACHE["dft"] = dft.astype(np.float32)
        _CACHE["ib"] = ib.astype(np.float32)
    return _CACHE["dft"], _CACHE["ib"]


def _sl(i, w=128):
    return slice(i * w, (i + 1) * w)


def _build_w2d(conv_w):
    w2d = np.zeros((96, D), np.float32)
    for k in range(3):
        for c in range(TF):
            w2d[32 * k + c] = conv_w[:, c, k]
    return w2d


def _hh(h):
    return slice(h * 512, (h + 1) * 512)


class K:
    def __init__(self):
        nc = bacc.Bacc()
        self.nc = nc
        p = nc.declare_dram_parameter
        self.d_xT = p("xT", [S * TF, N], F32, isOutput=False)
        self.d_w2d = p("w2d", [96, D], F32, isOutput=False)
        self.d_dft = p("dft", [N, 1024], F32, isOutput=False)
        self.d_ib = p("ib", [1024, N], F32, isOutput=False)
        self.d_idn = p("idn", [128, 128], F32, isOutput=False)
        self.d_e8 = p("e8", [HEADS, D], F32, isOutput=False)
        self.d_win = p("win", [L, D, D], F32, isOutput=False)
        self.d_wout = p("wout", [L, D, D], F32, isOutput=False)
        self.d_bin = p("binr", [L, D], F32, isOutput=False)
        self.d_bout = p("boutr", [L, 1, D], F32, isOutput=False)
        self.d_init = p("initf", [L, D], F32, isOutput=False)
        self.d_al8 = p("alpha8", [L, HEADS, 1], F32, isOutput=False)
        self.d_ffw1 = p("ffw1", [D, FD], F32, isOutput=False)
        self.d_ffb1 = p("ffb1", [1, FD], F32, isOutput=False)
        self.d_ffw2 = p("ffw2", [FD, D], F32, isOutput=False)
        self.d_ffb2 = p("ffb2", [1, D], F32, isOutput=False)
        self.d_convb = p("convb", [1, D], F32, isOutput=False)
        self.d_gpre = p("gprec", [D, 1], F32, isOutput=False)
        self.d_bpre = p("bprec", [D, 1], F32, isOutput=False)
        self.d_gpost = p("gpostr", [1, D], F32, isOutput=False)
        self.d_bpost = p("bpostr", [1, D], F32, isOutput=False)
        self.d_wg = p("lvwg", [L, D, TF], F32, isOutput=False)
        self.d_wp = p("lvwp", [L, D, TF], F32, isOutput=False)
        self.d_bg = p("lvbg", [L, 1, TF], F32, isOutput=False)
        self.d_bp = p("lvbp", [L, 1, TF], F32, isOutput=False)
        self.d_alv = p("lvalpha", [L, 1, 1], F32, isOutput=False)
        self.d_damp = p("damp8", [HEADS, 1], F32, isOutput=False)
        self.d_outw = p("outw", [D, TF], F32, isOutput=False)
        self.d_outb = p("outbr", [1, TF], F32, isOutput=False)
        self.d_out = p("outT", [S * TF, HOR], F32, isOutput=True)
        self.zmid = nc.dram_tensor("zmid", [S, N, D], F32)
        self.xtmid = nc.dram_tensor("xtmid", [S, TF, N], F32)

    # psum bank helper: tag-based reuse of the 8 banks
    def bank(self, i, shape=(128, 512)):
        tl = self.psp.tile(list(shape), F32, tag=f"bk{i}", name=f"bk{i}")
        return tl

    def build(self):
        nc = self.nc
        with ExitStack() as ctx:
            self.tc = ctx.enter_context(tile.TileContext(nc))
            tc = self.tc
            top = ctx.enter_context(tc.tile_pool(name="top", bufs=1))

            idn = top.tile([128, 128], F32, name="idn")
            nc.sync.dma_start(idn[:], self.d_idn[:])
            ones = top.tile([128, N], F32, name="ones")
            nc.vector.memset(ones[:], 1.0)
            e8 = top.tile([HEADS, D], F32, name="e8")
            nc.sync.dma_start(e8[:], self.d_e8[:])
            w2d = top.tile([96, D], F32, name="w2d")
            nc.sync.dma_start(w2d[:], self.d_w2d[:])
            # rows pack: p0 = ffb1[2048]; p32 = convb|gpost|bpost|ffb2 (4x512);
            # p64 = outb[7]
            rows = top.tile([128, FD], F32, name="rows")
            nc.sync.dma_start(rows[0:1, 0:FD], self.d_ffb1[:])
            nc.sync.dma_start(rows[32:33, 0:512], self.d_convb[:])
            nc.sync.dma_start(rows[32:33, 512:1024], self.d_gpost[:])
            nc.sync.dma_start(rows[32:33, 1024:1536], self.d_bpost[:])
            nc.sync.dma_start(rows[32:33, 1536:2048], self.d_ffb2[:])
            nc.sync.dma_start(rows[64:65, 0:TF], self.d_outb[:])
            # col pack: gpre(4) | bpre(4)
            cpk = top.tile([128, 8], F32, name="cpk")
            for dt in range(ND):
                nc.sync.dma_start(cpk[:, dt:dt + 1], self.d_gpre[_sl(dt), :])
                nc.sync.dma_start(cpk[:, 4 + dt:5 + dt], self.d_bpre[_sl(dt), :])
            outw = top.tile([128, ND * TF], F32, name="outw")
            for kt in range(ND):
                nc.sync.dma_start(outw[:, kt * TF:(kt + 1) * TF],
                                  self.d_outw[_sl(kt), :])
            eps = top.tile([128, 1], F32, name="eps")
            nc.vector.memset(eps[:], 1e-5)
            self.epst = eps
            agg = top.tile([128, S * ND * HOR], F32, name="agg")
            nc.vector.memset(agg[:], 0.0)
            csd = top.tile([128, ND * HOR], F32, name="csd")

            self.idn, self.ones, self.rows, self.cpk = idn, ones, rows, cpk
            self.e8t, self.w2dt_, self.aggt, self.csdt = e8, w2d, agg, csd
            self.outwt = outw

            with tc.tile_pool(name="ini", bufs=1) as ini, \
                    tc.tile_pool(name="inips", bufs=1, space="PSUM") as inips:
                self._damp_cs(ini, inips)

            for l in range(L):
                last = l == L - 1
                with tc.tile_pool(name=f"lay{l}", bufs=1) as layp, \
                        tc.tile_pool(name=f"wk{l}", bufs=1) as wk, \
                        tc.tile_pool(name=f"ps{l}", bufs=1, space="PSUM") as psp:
                    self.psp = psp
                    lay = self._layer_consts(l, layp)
                    for s in range(S):
                        self._sample(l, s, lay, wk)
                    if last:
                        for s in range(S):
                            self._output(s, wk)

        nc.compile()
        return nc

    # ---------- dampening cumsum -> csd [128, ND*HOR] ----------
    def _damp_cs(self, ini, inips):
        nc = self.nc
        ones = self.ones
        dcol = ini.tile([HEADS, 1], F32, name="dcol")
        nc.sync.dma_start(dcol[:], self.d_damp[:])
        df = ini.tile([HEADS, 1], F32, name="dfsig")
        nc.scalar.activation(df[:], dcol[:], AF.Sigmoid)
        dfb = ini.tile([HEADS, HOR], F32, name="dfb")
        nc.scalar.activation(dfb[:], ones[0:HEADS, 0:HOR], AF.Identity,
                             scale=df[:, 0:1])
        zer = ini.tile([HEADS, HOR], F32, name="zer8")
        nc.vector.memset(zer[:], 0.0)
        dfp = ini.tile([HEADS, HOR], F32, name="dfp")
        nc.vector.tensor_tensor_scan(dfp[:], dfb[:], zer[:], 1.0,
                                     op0=ALU.mult, op1=ALU.add)
        cs8 = ini.tile([HEADS, HOR], F32, name="cs8")
        nc.vector.tensor_tensor_scan(cs8[:], ones[0:HEADS, 0:HOR], dfp[:], 0.0,
                                     op0=ALU.mult, op1=ALU.add)
        for dt in range(ND):
            pini = inips.tile([128, HOR], F32, tag="pini", name="pini")
            nc.tensor.matmul(pini[:], self.e8t[:, _sl(dt)], cs8[:],
                             start=True, stop=True)
            nc.scalar.copy(self.csdt[:, dt * HOR:(dt + 1) * HOR], pini[:])

    # ---------- per-layer constants ----------
    def _layer_consts(self, l, layp):
        nc = self.nc
        ones = self.ones
        last = l == L - 1
        lay = {"l": l, "last": last}

        wdt = F32R if last else F32
        win = [layp.tile([128, D], wdt, name=f"win{k}") for k in range(ND)]
        wout = [layp.tile([128, D], wdt, name=f"wout{k}") for k in range(ND)]
        if last:
            for kt in range(ND):
                wtmp = layp.tile([128, D], F32, tag="wtmp", name="wtmp")
                nc.sync.dma_start(wtmp[:], self.d_win[l, _sl(kt), :])
                nc.vector.tensor_copy(win[kt][:], wtmp[:])
                wtmp2 = layp.tile([128, D], F32, tag="wtmp", name="wtmp")
                nc.sync.dma_start(wtmp2[:], self.d_wout[l, _sl(kt), :])
                nc.vector.tensor_copy(wout[kt][:], wtmp2[:])
        else:
            for kt in range(ND):
                nc.sync.dma_start(win[kt][:], self.d_win[l, _sl(kt), :])
                nc.sync.dma_start(wout[kt][:], self.d_wout[l, _sl(kt), :])

        # lrows: p0 = bout[512]; p32 = bg[7] then bp at cols 16..23
        lrows = layp.tile([128, 512], F32, name="lrows")
        nc.sync.dma_start(lrows[0:1, 0:D], self.d_bout[l, :, :])
        nc.sync.dma_start(lrows[32:33, 0:TF], self.d_bg[l, :, :])
        nc.sync.dma_start(lrows[32:33, 16:16 + TF], self.d_bp[l, :, :])

        # lcol pack [128, 16]: al(4) oma(4) init(4) bi(4); plus lv cols [7,1]
        lcol = layp.tile([128, 24], F32, name="lcol")
        al8 = layp.tile([HEADS, 1], F32, tag="al8t", name="al8")
        nc.sync.dma_start(al8[:], self.d_al8[l, :, :])
        al8s = layp.tile([HEADS, 1], F32, tag="al8s", name="al8s")
        nc.scalar.activation(al8s[:], al8[:], AF.Sigmoid)
        for dt in range(ND):
            pal = self.psp.tile([128, 1], F32, tag="bk0", name="pal")
            nc.tensor.matmul(pal[:], self.e8t[:, _sl(dt)], al8s[:],
                             start=True, stop=True)
            nc.scalar.copy(lcol[:, dt:dt + 1], pal[:])
        for dt in range(ND):
            nc.vector.tensor_scalar(lcol[:, 4 + dt:5 + dt], lcol[:, dt:dt + 1],
                                    -1.0, 1.0, op0=ALU.mult, op1=ALU.add)
            nc.sync.dma_start(lcol[:, 8 + dt:9 + dt],
                              self.d_init[l, _sl(dt)].rearrange("(a b) -> a b", b=1))
        bi = layp.tile([128, ND], F32, tag="bitmp", name="bitmp")
        for dt in range(ND):
            nc.sync.dma_start(bi[:, dt:dt + 1],
                              self.d_bin[l, _sl(dt)].rearrange("(a b) -> a b", b=1))
        nc.vector.tensor_sub(lcol[:, 12:16], bi[:], lcol[:, 8:12])
        # level alpha
        alv = layp.tile([1, 1], F32, tag="alvt", name="alv")
        nc.sync.dma_start(alv[:], self.d_alv[l, :, :])
        alvs = layp.tile([1, 1], F32, tag="alvst", name="alvs")
        nc.scalar.activation(alvs[:], alv[:], AF.Sigmoid)
        pv = self.psp.tile([TF, 1], F32, tag="bk1", name="palv")
        nc.tensor.matmul(pv[:], ones[0:1, 0:TF], alvs[:], start=True, stop=True)
        nc.scalar.copy(lcol[0:TF, 16:17], pv[:])
        nc.vector.tensor_scalar(lcol[0:TF, 17:18], lcol[0:TF, 16:17], -1.0, 1.0,
                                op0=ALU.mult, op1=ALU.add)

        # level weights [128, TF] x4 packed [128, 2*ND*TF]
        lw = layp.tile([128, 2 * ND * TF], F32, name="lw")
        for kt in range(ND):
            nc.sync.dma_start(lw[:, kt * TF:(kt + 1) * TF], self.d_wg[l, _sl(kt), :])
            nc.sync.dma_start(lw[:, (ND + kt) * TF:(ND + kt + 1) * TF],
                              self.d_wp[l, _sl(kt), :])

        lay.update(win=win, wout=wout, lrows=lrows, lcol=lcol, lw=lw)
        return lay

    # ---------- one sample through one layer ----------
    def _sample(self, l, s, lay, wk):
        nc = self.nc
        ones, idn = self.ones, self.idn
        last = lay["last"]
        agg = self.aggt

        def aggsl(dt):
            return self.aggt[:, (s * ND + dt) * HOR:(s * ND + dt + 1) * HOR]

        # --- z input: conv (l0) or reload (l1)
        z = [wk.tile([128, D], F32, tag=f"B1_{tt}", name=f"z{tt}")
             for tt in range(NT)]
        if l == 0:
            xsh = wk.tile([96, N], F32, tag="xsh", name="xsh")
            xts = wk.tile([TF, N], F32, tag="xts", name="xts")
            nc.sync.dma_start(xts[:], self.d_xT[s * TF:(s + 1) * TF, :])
            nc.vector.memset(xsh[:], 0.0)
            nc.vector.tensor_copy(xsh[0:TF, 1:N], xts[:, 0:N - 1])
            nc.vector.tensor_copy(xsh[32:32 + TF, 0:N], xts[:, 0:N])
            nc.vector.tensor_copy(xsh[64:64 + TF, 0:N - 1], xts[:, 1:N])
            for tt in range(NT):
                pz = self.bank(tt % 2)
                nc.tensor.matmul(pz[:], xsh[:, _sl(tt)], self.w2dt_[:],
                                 start=True, stop=False)
                nc.tensor.matmul(pz[:], ones[32:33, 0:128], self.rows[32:33, 0:512],
                                 start=False, stop=True)
                nc.scalar.copy(z[tt][:], pz[:])
        else:
            for tt in range(NT):
                nc.sync.dma_start(z[tt][:], self.zmid[s, _sl(tt), :])

        # --- rfft (fp32, dft streamed, 8 psum banks)
        psA = [self.bank(ct) for ct in range(ND)]
        psB = [self.bank(4 + ct) for ct in range(ND)]
        for kt in range(NT):
            dftk = wk.tile([128, 1024], F32, tag=f"dftk{kt % 2}", name="dftk")
            nc.sync.dma_start(dftk[:], self.d_dft[_sl(kt), :])
            for ct in range(ND):
                nc.tensor.matmul(psA[ct][:], z[kt][:, _sl(ct)], dftk[:, 0:512],
                                 start=(kt == 0), stop=(kt == NT - 1))
                nc.tensor.matmul(psB[ct][:], z[kt][:, _sl(ct)], dftk[:, 512:1024],
                                 start=(kt == 0), stop=(kt == NT - 1))

        # --- top-4 mask -> filt [ND][128, 1024] ([c, f])
        filt = [wk.tile([128, 1024], F32, tag=f"A1_{ct}", name=f"filt{ct}")
                for ct in range(ND)]
        for ct in range(ND):
            sqA = wk.tile([128, 512], F32, tag="sqA", name="sqA")
            nc.scalar.activation(sqA[:], psA[ct][:], AF.Square)
            sqB = wk.tile([128, 512], F32, tag="sqB", name="sqB")
            nc.scalar.activation(sqB[:], psB[ct][:], AF.Square)
            amp2 = wk.tile([128, 513], F32, tag="amp2", name="amp2")
            nc.vector.tensor_add(amp2[:, 1:512], sqA[:, 1:512], sqB[:, 1:512])
            nc.scalar.copy(amp2[:, 0:1], sqA[:, 0:1])
            nc.scalar.copy(amp2[:, 512:513], sqB[:, 0:1])
            top8 = wk.tile([128, 8], F32, tag="top8", name="top8")
            nc.vector.max(top8[:], amp2[:])
            kth = top8[:, 3:4]
            nc.vector.scalar_tensor_tensor(filt[ct][:, 0:512], amp2[:, 0:512],
                                           kth, psA[ct][:],
                                           op0=ALU.is_ge, op1=ALU.mult)
            nc.vector.scalar_tensor_tensor(filt[ct][:, 513:1024], amp2[:, 1:512],
                                           kth, psB[ct][:, 1:512],
                                           op0=ALU.is_ge, op1=ALU.mult)
            nc.vector.scalar_tensor_tensor(filt[ct][:, 512:513], amp2[:, 512:513],
                                           kth, psB[ct][:, 0:1],
                                           op0=ALU.is_ge, op1=ALU.mult)

        # --- transpose filt -> filtT [NT][128, 512] ([f, c])
        fdt = F32R if last else F32
        filtT = [wk.tile([128, 512], fdt, tag=f"B2_{ft}", name=f"filtT{ft}")
                 for ft in range(NT)]
        for ft in range(NT):
            pT = self.bank(ft % 2)
            for ct in range(ND):
                nc.tensor.transpose(pT[:, _sl(ct)], filt[ct][:, _sl(ft)], idn[:])
            if last:
                nc.vector.tensor_copy(filtT[ft][:], pT[:])
            else:
                nc.scalar.copy(filtT[ft][:], pT[:])

        # --- irfft (ib streamed, 8 banks) -> lp, z2
        pl = [self.bank(tt) for tt in range(NT)]
        for ft in range(NT):
            ibk = wk.tile([128, 1024], fdt, tag=f"ibk{ft % 2}", name="ibk")
            if last:
                ibf = wk.tile([128, 1024], F32, tag="ibf", name="ibf")
                nc.sync.dma_start(ibf[:], self.d_ib[_sl(ft), :])
                nc.vector.tensor_copy(ibk[:], ibf[:])
            else:
                nc.sync.dma_start(ibk[:], self.d_ib[_sl(ft), :])
            for tt in range(NT):
                nc.tensor.matmul(pl[tt][:], ibk[:, _sl(tt)], filtT[ft][:],
                                 start=(ft == 0), stop=(ft == NT - 1))
        lp = [wk.tile([128, D], F32, tag=f"B3_{tt}", name=f"lp{tt}")
              for tt in range(NT)]
        z2 = [wk.tile([128, D], F32, tag=f"B4_{tt}", name=f"z2_{tt}")
              for tt in range(NT)]
        for tt in range(NT):
            nc.scalar.copy(lp[tt][:], pl[tt][:])
            nc.vector.tensor_sub(z2[tt][:], z[tt][:], pl[tt][:])

        # --- lpT [ND][128, N] (tag A2) + extrap + perT; then free
        lpT = [wk.tile([128, N], F32, tag=f"A2_{dt}", name=f"lpT{dt}")
               for dt in range(ND)]
        for dt in range(ND):
            for h in range(2):
                pT = self.bank(dt % 2)
                for q in range(4):
                    nc.tensor.transpose(pT[:, _sl(q)], lp[h * 4 + q][:, _sl(dt)],
                                        idn[:])
                nc.scalar.copy(lpT[dt][:, _hh(h)], pT[:])
            nc.vector.tensor_add(aggsl(dt), aggsl(dt), lpT[dt][:, 0:HOR])
        perT = wk.tile([TF, N], F32, tag="perT", name="perT")
        for h in range(2):
            pp = self.bank(2)
            for kt in range(ND):
                nc.tensor.matmul(pp[0:TF, :], lay["lw"][:, (ND + kt) * TF:(ND + kt + 1) * TF],
                                 lpT[kt][:, _hh(h)], start=(kt == 0), stop=False)
            nc.tensor.matmul(pp[0:TF, :], lay["lrows"][32:33, 16:16 + TF],
                             ones[32:33, 0:512], start=False, stop=True)
            nc.scalar.copy(perT[:, _hh(h)], pp[0:TF, :])

        # --- z2T (tag A2 reuse after lpT dead)
        zdt = F32R if last else F32
        z2T = [wk.tile([128, N], zdt, tag=f"A2_{dt}", name=f"z2T{dt}")
               for dt in range(ND)]
        for dt in range(ND):
            for h in range(2):
                pT = self.bank(dt % 2)
                for q in range(4):
                    nc.tensor.transpose(pT[:, _sl(q)], z2[h * 4 + q][:, _sl(dt)],
                                        idn[:])
                if last:
                    nc.vector.tensor_copy(z2T[dt][:, _hh(h)], pT[:])
                else:
                    nc.scalar.copy(z2T[dt][:, _hh(h)], pT[:])

        # --- win GEMM -> xinT (tag A1 reuse: filt dead)
        xinT = [wk.tile([128, N], F32, tag=f"A1_{dt}", name=f"xinT{dt}")
                for dt in range(ND)]
        for dt in range(ND):
            for h in range(2):
                px = self.bank(4 + dt % 2)
                for kt in range(ND):
                    nc.tensor.matmul(px[:], lay["win"][kt][:, _sl(dt)],
                                     z2T[kt][:, _hh(h)],
                                     start=(kt == 0), stop=(kt == ND - 1))
                nc.scalar.copy(xinT[dt][:, _hh(h)], px[:])

        # --- xd -> scan -> sT (tag A2 reuse: z2T dead)
        sdt = F32R if last else F32
        sT = [wk.tile([128, N], sdt, tag=f"A2_{dt}", name=f"sT{dt}")
              for dt in range(ND)]
        lc = lay["lcol"]
        for dt in range(ND):
            xd = wk.tile([128, N], F32, tag="xd", name="xd")
            nc.vector.tensor_sub(xd[:, 1:N], xinT[dt][:, 1:N], xinT[dt][:, 0:N - 1])
            nc.vector.tensor_scalar_add(xd[:, 0:1], xinT[dt][:, 0:1],
                                        lc[:, 12 + dt:13 + dt])
            nc.vector.tensor_scalar_mul(xd[:], xd[:], lc[:, dt:dt + 1])
            if OMA_BCAST:
                omab_ap = lc[:, 4 + dt:5 + dt].broadcast_to([128, N])
            else:
                omab = wk.tile([128, N], F32, tag="omab", name="omab")
                nc.vector.memset(omab[:], 1.0)
                nc.vector.tensor_scalar_mul(omab[:], omab[:], lc[:, 4 + dt:5 + dt])
                omab_ap = omab[:]
            nc.vector.tensor_tensor_scan(sT[dt][:], omab_ap, xd[:],
                                         lc[:, 8 + dt:9 + dt],
                                         op0=ALU.mult, op1=ALU.add)

        # --- wout GEMM -> lg [t,d] (tag B2 reuse: filtT dead) (+ z3 if l0)
        lg = [wk.tile([128, D], F32, tag=f"B2_{tt}", name=f"lg{tt}")
              for tt in range(NT)]
        for tt in range(NT):
            pg = self.bank(tt % 2)
            for kt in range(ND):
                nc.tensor.matmul(pg[:], sT[kt][:, _sl(tt)], lay["wout"][kt][:],
                                 start=(kt == 0), stop=False)
            nc.tensor.matmul(pg[:], ones[0:1, 0:128], lay["lrows"][0:1, 0:D],
                             start=False, stop=True)
            nc.scalar.copy(lg[tt][:], pg[:])
            if not last:
                # z3 overwrites z (tag B1): z dead after z2
                nc.vector.tensor_sub(z[tt][:], z2[tt][:], pg[:])
        z3 = z

        # --- lgT via transposes (tag A1 reuse: xinT dead)
        lgT = [wk.tile([128, N], F32, tag=f"A1_{dt}", name=f"lgT{dt}")
               for dt in range(ND)]
        for dt in range(ND):
            for h in range(2):
                pT = self.bank(2 + dt % 2)
                for q in range(4):
                    nc.tensor.transpose(pT[:, _sl(q)], lg[h * 4 + q][:, _sl(dt)],
                                        idn[:])
                nc.scalar.copy(lgT[dt][:, _hh(h)], pT[:])
            # damp: agg += lg_last * csd
            nc.vector.scalar_tensor_tensor(
                aggsl(dt), self.csdt[:, dt * HOR:(dt + 1) * HOR],
                lgT[dt][:, N - 1:N], aggsl(dt), op0=ALU.mult, op1=ALU.add)

        # --- level: grT; scans update xtmid
        grT = wk.tile([TF, N], F32, tag="grT", name="grT")
        for h in range(2):
            pgr = self.bank(6)
            for kt in range(ND):
                nc.tensor.matmul(pgr[0:TF, :], lay["lw"][:, kt * TF:(kt + 1) * TF],
                                 lgT[kt][:, _hh(h)], start=(kt == 0), stop=False)
            nc.tensor.matmul(pgr[0:TF, :], lay["lrows"][32:33, 0:TF],
                             ones[32:33, 0:512], start=False, stop=True)
            nc.scalar.copy(grT[:, _hh(h)], pgr[0:TF, :])

        xts2 = wk.tile([TF, N], F32, tag="xts", name="xts2")
        if l == 0:
            nc.sync.dma_start(xts2[:], self.d_xT[s * TF:(s + 1) * TF, :])
        else:
            nc.sync.dma_start(xts2[:], self.xtmid[s, :, :])
        v = wk.tile([TF, N], F32, tag="lvv", name="lvv")
        nc.vector.tensor_sub(v[:], xts2[:], perT[:])
        nc.vector.tensor_scalar_mul(v[:], v[:], lc[0:TF, 16:17])
        if OMA_BCAST:
            omlv_ap = lc[0:TF, 17:18].broadcast_to([TF, N])
        else:
            omlv = wk.tile([TF, N], F32, tag="omlv", name="omlv")
            nc.vector.memset(omlv[:], 1.0)
            nc.vector.tensor_scalar_mul(omlv[:], omlv[:], lc[0:TF, 17:18])
            omlv_ap = omlv[:]
        pt = wk.tile([TF, N], F32, tag="lvp", name="lvp")
        nc.vector.tensor_tensor_scan(pt[:], omlv_ap, v[:], 0.0,
                                     op0=ALU.mult, op1=ALU.add)
        gt = wk.tile([TF, N], F32, tag="lvv", name="lvg")
        nc.vector.tensor_tensor_scan(gt[:], omlv_ap, grT[:], 0.0,
                                     op0=ALU.mult, op1=ALU.add)
        xnew = wk.tile([TF, N], F32, tag="grT", name="xnew")
        nc.vector.tensor_add(xnew[:], pt[:], gt[:])
        nc.sync.dma_start(self.xtmid[s, :, :], xnew[:])

        # --- FF (layer 0 only), then spill z4
        if not last:
            z4 = self._ff(s, z3, wk)
            for tt in range(NT):
                nc.sync.dma_start(self.zmid[s, _sl(tt), :], z4[tt][:])

    # ---------- LN stats ----------
    def _ln_stats(self, zset, wk, tagp):
        nc = self.nc
        st = wk.tile([128, 8 * NT], F32, tag=f"st{tagp}", name=f"st{tagp}")
        mu8 = st[:, 0:NT]
        s28 = st[:, NT:2 * NT]
        scr = wk.tile([128, D], F32, tag="lnscr", name="lnscr")
        for tt in range(NT):
            nc.vector.tensor_reduce(st[:, tt:tt + 1], zset[tt][:],
                                    mybir.AxisListType.X, op=ALU.add)
            nc.scalar.activation(scr[:], zset[tt][:], AF.Square,
                                 accum_out=st[:, NT + tt:NT + tt + 1])
        mun = st[:, 2 * NT:3 * NT]
        nc.vector.tensor_scalar_mul(mun, mu8, 1.0 / D)
        ex2 = st[:, 3 * NT:4 * NT]
        nc.vector.tensor_scalar_mul(ex2, s28, 1.0 / D)
        musq = st[:, 4 * NT:5 * NT]
        nc.scalar.activation(musq, mun, AF.Square)
        var = st[:, 5 * NT:6 * NT]
        nc.vector.tensor_sub(var, ex2, musq)
        sd = st[:, 6 * NT:7 * NT]
        nc.scalar.activation(sd, var, AF.Sqrt, bias=self.epst[:, 0:1])
        rs = st[:, 7 * NT:8 * NT]
        nc.vector.reciprocal(rs, sd)
        nmurs = st[:, 4 * NT:5 * NT]  # overwrite musq slot
        nc.vector.tensor_mul(nmurs, mun, rs)
        nc.vector.tensor_scalar_mul(nmurs, nmurs, -1.0)
        return rs, nmurs

    # ---------- FF block ----------
    def _ff(self, s, z3, wk):
        nc = self.nc
        ones, idn = self.ones, self.idn
        rows, cpk = self.rows, self.cpk
        rs, nmurs = self._ln_stats(z3, wk, "pre")
        # h = (z3-mu)*rs, overwrite z3 tiles in place via scratch
        h_ = [wk.tile([128, D], F32, tag=f"B2_{tt}", name=f"h{tt}")
              for tt in range(NT)]
        for tt in range(NT):
            nc.scalar.activation(h_[tt][:], z3[tt][:], AF.Identity,
                                 scale=rs[:, tt:tt + 1], bias=nmurs[:, tt:tt + 1])
        hT = [wk.tile([128, N], F32, tag=f"A2_{dt}", name=f"hT{dt}")
              for dt in range(ND)]
        for dt in range(ND):
            for h in range(2):
                pT = self.bank(dt % 2)
                for q in range(4):
                    nc.tensor.transpose(pT[:, _sl(q)], h_[h * 4 + q][:, _sl(dt)],
                                        idn[:])
                nc.scalar.copy(hT[dt][:, _hh(h)], pT[:])
        znT = [wk.tile([128, N], F32, tag=f"A1_{dt}", name=f"znT{dt}")
               for dt in range(ND)]
        for dt in range(ND):
            nc.scalar.activation(znT[dt][:], hT[dt][:], AF.Identity,
                                 scale=cpk[:, dt:dt + 1], bias=cpk[:, 4 + dt:5 + dt])

        yT = [wk.tile([128, N], F32, tag=f"A2_{dt}", name=f"yT{dt}")
              for dt in range(ND)]
        for h in range(2):
            pzf = [self.bank(dt) for dt in range(ND)]
            for m in range(NM):
                w1m = wk.tile([128, ND * 128], F32, tag=f"w1m{m % 2}", name="w1m")
                for kt in range(ND):
                    nc.sync.dma_start(w1m[:, _sl(kt)], self.d_ffw1[_sl(kt), _sl(m)])
                ph = self.bank(4 + m % 2)
                for kt in range(ND):
                    nc.tensor.matmul(ph[:], w1m[:, _sl(kt)], znT[kt][:, _hh(h)],
                                     start=(kt == 0), stop=False)
                nc.tensor.matmul(ph[:], rows[0:1, _sl(m)], ones[0:1, 0:512],
                                 start=False, stop=True)
                sig = wk.tile([128, 512], F32, tag=f"sig{m % 2}", name="sig")
                nc.scalar.activation(sig[:], ph[:], AF.Sigmoid)
                w2m = wk.tile([128, 512], F32, tag=f"w2m{m % 2}", name="w2m")
                nc.sync.dma_start(w2m[:], self.d_ffw2[_sl(m), :])
                for dt in range(ND):
                    nc.tensor.matmul(pzf[dt][:], w2m[:, _sl(dt)], sig[:],
                                     start=(m == 0), stop=False)
            for dt in range(ND):
                nc.tensor.matmul(pzf[dt][:], rows[32:33, 1536 + dt * 128:1536 + (dt + 1) * 128],
                                 ones[32:33, 0:512], start=False, stop=True)
                nc.vector.tensor_add(yT[dt][:, _hh(h)], pzf[dt][:],
                                     znT[dt][:, _hh(h)])

        y = [wk.tile([128, D], F32, tag=f"B4_{tt}", name=f"y{tt}")
             for tt in range(NT)]
        for tt in range(NT):
            pT = self.bank(6)
            for dt in range(ND):
                nc.tensor.transpose(pT[:, _sl(dt)], yT[dt][:, _sl(tt)], idn[:])
            nc.scalar.copy(y[tt][:], pT[:])

        rs2, nmurs2 = self._ln_stats(y, wk, "post")
        gb = wk.tile([128, D], F32, tag="gpb", name="gpb")
        pb = self.bank(0)
        nc.tensor.matmul(pb[:], ones[32:33, 0:128], rows[32:33, 512:1024],
                         start=True, stop=True)
        nc.scalar.copy(gb[:], pb[:])
        bb = wk.tile([128, D], F32, tag="bpb", name="bpb")
        pb2 = self.bank(1)
        nc.tensor.matmul(pb2[:], ones[32:33, 0:128], rows[32:33, 1024:1536],
                         start=True, stop=True)
        nc.scalar.copy(bb[:], pb2[:])
        z4 = [wk.tile([128, D], F32, tag=f"B1_{tt}", name=f"z4_{tt}")
              for tt in range(NT)]
        scr2 = wk.tile([128, D], F32, tag="lnscr", name="scr2")
        for tt in range(NT):
            nc.scalar.activation(scr2[:], y[tt][:], AF.Identity,
                                 scale=rs2[:, tt:tt + 1], bias=nmurs2[:, tt:tt + 1])
            nc.vector.tensor_mul(z4[tt][:], scr2[:], gb[:])
            nc.vector.tensor_add(z4[tt][:], z4[tt][:], bb[:])
        return z4

    # ---------- output head ----------
    def _output(self, s, wk):
        nc = self.nc
        ones = self.ones
        po = self.bank(7)
        for kt in range(ND):
            nc.tensor.matmul(po[0:TF, 0:HOR], self.outwt[:, kt * TF:(kt + 1) * TF],
                             self.aggt[:, (s * ND + kt) * HOR:(s * ND + kt + 1) * HOR],
                             start=(kt == 0), stop=False)
        nc.tensor.matmul(po[0:TF, 0:HOR], self.rows[64:65, 0:TF],
                         ones[64:65, 0:HOR], start=False, stop=True)
        xfin = wk.tile([TF, N], F32, tag="xts", name="xfin")
        nc.sync.dma_start(xfin[:], self.xtmid[s, :, :])
        oT = wk.tile([TF, HOR], F32, tag="oT", name="oT")
        nc.vector.tensor_scalar_add(oT[:], po[0:TF, 0:HOR], xfin[:, N - 1:N])
        nc.sync.dma_start(self.d_out[s * TF:(s + 1) * TF, :], oT[:])


def _get_nc():
    if "nc" not in _CACHE:
        _CACHE["nc"] = K().build()
    return _CACHE["nc"]


def _common_maps(inputs, w2d, dft, ib, e8):
    return dict(
        w2d=w2d,
        convb=np.asarray(inputs["conv_b"], np.float32).reshape(1, D),
        dft=dft, ib=ib,
        idn=np.eye(128, dtype=np.float32),
        e8=e8,
        win=np.asarray(inputs["mhesa_win"], np.float32),
        wout=np.asarray(inputs["mhesa_wout"], np.float32),
        binr=np.asarray(inputs["mhesa_bin"], np.float32),
        boutr=np.asarray(inputs["mhesa_bout"], np.float32).reshape(L, 1, D),
        initf=np.asarray(inputs["mhesa_init"], np.float32).reshape(L, D),
        alpha8=np.asarray(inputs["mhesa_alpha"], np.float32).reshape(L, HEADS, 1),
        ffw1=np.asarray(inputs["ff_w1"], np.float32),
        ffb1=np.asarray(inputs["ff_b1"], np.float32).reshape(1, FD),
        ffw2=np.asarray(inputs["ff_w2"], np.float32),
        ffb2=np.asarray(inputs["ff_b2"], np.float32).reshape(1, D),
        gprec=np.asarray(inputs["ff_pre_g"], np.float32).reshape(D, 1),
        bprec=np.asarray(inputs["ff_pre_b"], np.float32).reshape(D, 1),
        gpostr=np.asarray(inputs["ff_post_g"], np.float32).reshape(1, D),
        bpostr=np.asarray(inputs["ff_post_b"], np.float32).reshape(1, D),
        lvwg=np.asarray(inputs["level_wg"], np.float32),
        lvwp=np.asarray(inputs["level_wp"], np.float32),
        lvbg=np.asarray(inputs["level_bg"], np.float32).reshape(L, 1, TF),
        lvbp=np.asarray(inputs["level_bp"], np.float32).reshape(L, 1, TF),
        lvalpha=np.asarray(inputs["level_alpha"], np.float32).reshape(L, 1, 1),
        damp8=np.asarray(inputs["dampen_factor"], np.float32).reshape(HEADS, 1),
        outw=np.asarray(inputs["out_w"], np.float32),
        outbr=np.asarray(inputs["out_b"], np.float32).reshape(1, TF),
    )


def _kernel_impl(inputs, runner):
    x = np.asarray(inputs["x"], np.float32)
    assert (x.shape[0], x.shape[1], x.shape[2]) == (32, N, TF)
    assert int(inputs["forecast_horizon"]) == HOR
    dft, ib = _dft_consts()
    conv_w = np.asarray(inputs["conv_w"], np.float32)
    w2d = _build_w2d(conv_w)
    e8 = np.repeat(np.eye(HEADS, dtype=np.float32), DH, axis=1)
    nc = _get_nc()
    common = _common_maps(inputs, w2d, dft, ib, e8)
    in_maps = []
    for c in range(NCORES):
        xs = x[c * S:(c + 1) * S]
        xT = xs.transpose(0, 2, 1).reshape(S * TF, N).copy()
        in_maps.append(dict(common, xT=xT))
    res = runner(nc, in_maps)
    out = np.zeros((x.shape[0], HOR, TF), np.float32)
    for c in range(NCORES):
        oT = res.results[c]["outT"].reshape(S, TF, HOR)
        out[c * S:(c + 1) * S] = oT.transpose(0, 2, 1)
    return out, res


def kernel(**inputs):
    out, _ = _kernel_impl(
        inputs,
        lambda nc, im: run_bass_kernel_spmd(nc, im, list(range(NCORES))))
    return out


def kernel_traced(**inputs):
    """Like kernel() but with NTFF profiling; returns (out, BassKernelResults)."""
    return _kernel_impl(
        inputs,
        lambda nc, im: run_bass_kernel_spmd(nc, im, list(range(NCORES)),
                                            trace=True))


# revision 15
# speedup vs baseline: 29.0232x; 29.0232x over previous
"""ETSFormer forward pass on 8 Trainium2 NeuronCores (Bass/Tile).

Data-parallel over batch: 32 samples -> 8 cores x 4 samples, weights
replicated, no collectives. The reference's FFT machinery is computed
exactly without FFTs:
  - freq_attention: dense DFT matmuls + hardware top-8 (vector.max) mask
  - mhesa / level exponential smoothing: the reference FFT cross-correlation
    is exactly a first-order EMA -> hardware prefix scan (tensor_tensor_scan)
  - fourier_extrapolate: exact slice (Dirichlet kernel identity)

Precision: the top-4 frequency mask is extremely sensitive (2e-4 relative
amp noise -> 2.6e-2 output error), so every GEMM feeding a ranking (conv,
rfft both layers, irfft/mhesa/FF of layer 0) runs in fp32; post-ranking
paths (layer-1 irfft/mhesa via lgT, level, damp, output head) run fp32r.
"""
import numpy as np
from contextlib import ExitStack

import concourse.bass as bass
import concourse.bacc as bacc
import concourse.tile as tile
from concourse import mybir
from concourse.bass_utils import run_bass_kernel_spmd

F32 = mybir.dt.float32
F32R = mybir.dt.float32r
AF = mybir.ActivationFunctionType
ALU = mybir.AluOpType

N = 1024
D = 512
TF = 7
HEADS = 8
DH = D // HEADS
L = 2
S = 4
NCORES = 8
HOR = 96
FD = 2048
NT = N // 128   # 8
ND = D // 128   # 4
NM = FD // 128  # 16

_CACHE = {}
OMA_BCAST = True


def _dft_consts():
    if "dft" not in _CACHE:
        t = np.arange(N)
        f = np.arange(513)
        ang = 2.0 * np.pi * np.outer(t, f) / N
        cos = np.cos(ang)
        sin = np.sin(ang)
        dft = np.zeros((N, 1024), np.float64)
        dft[:, 0:512] = cos[:, 0:512]
        dft[:, 512] = cos[:, 512]
        dft[:, 513:1024] = sin[:, 1:512]
        c = np.full(513, 2.0)
        c[0] = 1.0
        c[512] = 1.0
        ib = np.zeros((1024, N), np.float64)
        ib[0:512, :] = (c[0:512, None] / N) * cos[:, 0:512].T
        ib[512, :] = (1.0 / N) * cos[:, 512]
        ib[513:1024, :] = (2.0 / N) * sin[:, 1:512].T
        _CACHE["dft"] = dft.astype(np.float32)
        _CACHE["ib"] = ib.astype(np.float32)
    return _CACHE["dft"], _CACHE["ib"]


def _sl(i, w=128):
    return slice(i * w, (i + 1) * w)


def _build_w2d(conv_w):
    w2d = np.zeros((96, D), np.float32)
    for k in range(3):
        for c in range(TF):
            w2d[32 * k + c] = conv_w[:, c, k]
    return w2d


def _hh(h):
    return slice(h * 512, (h + 1) * 512)


class K:
    def __init__(self):
        nc = bacc.Bacc()
        self.nc = nc
        p = nc.declare_dram_parameter
        self.d_xT = p("xT", [S * TF, N], F32, isOutput=False)
        self.d_w2d = p("w2d", [96, D], F32, isOutput=False)
        self.d_dft = p("dft", [N, 1024], F32, isOutput=False)
        self.d_ib = p("ib", [1024, N], F32, isOutput=False)
        self.d_idn = p("idn", [128, 128], F32, isOutput=False)
        self.d_e8 = p("e8", [HEADS, D], F32, isOutput=False)
        self.d_win = p("win", [L, D, D], F32, isOutput=False)
        self.d_wout = p("wout", [L, D, D], F32, isOutput=False)
        self.d_bin = p("binr", [L, D], F32, isOutput=False)
        self.d_bout = p("boutr", [L, 1, D], F32, isOutput=False)
        self.d_init = p("initf", [L, D], F32, isOutput=False)
        self.d_al8 = p("alpha8", [L, HEADS, 1], F32, isOutput=False)
        self.d_ffw1 = p("ffw1", [D, FD], F32, isOutput=False)
        self.d_ffb1 = p("ffb1", [FD, 1], F32, isOutput=False)
        self.d_ffw2 = p("ffw2", [FD, D], F32, isOutput=False)
        self.d_ffb2 = p("ffb2", [D, 1], F32, isOutput=False)
        self.d_convb = p("convb", [1, D], F32, isOutput=False)
        self.d_gpre = p("gprec", [D, 1], F32, isOutput=False)
        self.d_bpre = p("bprec", [D, 1], F32, isOutput=False)
        self.d_gpost = p("gpostr", [1, D], F32, isOutput=False)
        self.d_bpost = p("bpostr", [1, D], F32, isOutput=False)
        self.d_wg = p("lvwg", [L, D, TF], F32, isOutput=False)
        self.d_wp = p("lvwp", [L, D, TF], F32, isOutput=False)
        self.d_bg = p("lvbg", [L, 1, TF], F32, isOutput=False)
        self.d_bp = p("lvbp", [L, 1, TF], F32, isOutput=False)
        self.d_alv = p("lvalpha", [L, 1, 1], F32, isOutput=False)
        self.d_damp = p("damp8", [HEADS, 1], F32, isOutput=False)
        self.d_outw = p("outw", [D, TF], F32, isOutput=False)
        self.d_outb = p("outbr", [1, TF], F32, isOutput=False)
        self.d_out = p("outT", [S * TF, HOR], F32, isOutput=True)
        self.zmid = nc.dram_tensor("zmid", [S, N, D], F32)
        self.xtmid = nc.dram_tensor("xtmid", [S, TF, N], F32)

    # psum bank helper: tag-based reuse of the 8 banks
    def bank(self, i, shape=(128, 512)):
        tl = self.psp.tile(list(shape), F32, tag=f"bk{i}", name=f"bk{i}")
        return tl

    def build(self):
        nc = self.nc
        with ExitStack() as ctx:
            self.tc = ctx.enter_context(tile.TileContext(nc))
            tc = self.tc
            top = ctx.enter_context(tc.tile_pool(name="top", bufs=1))

            idn = top.tile([128, 128], F32, name="idn")
            nc.sync.dma_start(idn[:], self.d_idn[:])
            ones = top.tile([128, N], F32, name="ones")
            nc.vector.memset(ones[:], 1.0)
            e8 = top.tile([HEADS, D], F32, name="e8")
            nc.sync.dma_start(e8[:], self.d_e8[:])
            w2d = top.tile([96, D], F32, name="w2d")
            nc.sync.dma_start(w2d[:], self.d_w2d[:])
            # rows pack: p0 = ffb1[2048]; p32 = convb|gpost|bpost|ffb2 (4x512);
            # p64 = outb[7]
            rows = top.tile([128, FD], F32, name="rows")
            nc.sync.dma_start(rows[32:33, 0:512], self.d_convb[:])
            nc.sync.dma_start(rows[32:33, 512:1024], self.d_gpost[:])
            nc.sync.dma_start(rows[32:33, 1024:1536], self.d_bpost[:])
            nc.sync.dma_start(rows[64:65, 0:TF], self.d_outb[:])
            # col pack: gpre(4) | bpre(4)
            cpk = top.tile([128, 28], F32, name="cpk")
            for dt in range(ND):
                nc.sync.dma_start(cpk[:, dt:dt + 1], self.d_gpre[_sl(dt), :])
                nc.sync.dma_start(cpk[:, 4 + dt:5 + dt], self.d_bpre[_sl(dt), :])
            for m in range(NM):
                nc.sync.dma_start(cpk[:, 8 + m:9 + m], self.d_ffb1[_sl(m), :])
            for dt in range(ND):
                nc.sync.dma_start(cpk[:, 24 + dt:25 + dt],
                                  self.d_ffb2[_sl(dt), :])
            outw = top.tile([128, ND * TF], F32, name="outw")
            for kt in range(ND):
                nc.sync.dma_start(outw[:, kt * TF:(kt + 1) * TF],
                                  self.d_outw[_sl(kt), :])
            eps = top.tile([128, 1], F32, name="eps")
            nc.vector.memset(eps[:], 1e-5)
            self.epst = eps
            agg = top.tile([128, S * ND * HOR], F32, name="agg")
            nc.vector.memset(agg[:], 0.0)
            csd = top.tile([128, ND * HOR], F32, name="csd")

            self.idn, self.ones, self.rows, self.cpk = idn, ones, rows, cpk
            self.e8t, self.w2dt_, self.aggt, self.csdt = e8, w2d, agg, csd
            self.outwt = outw

            with tc.tile_pool(name="ini", bufs=1) as ini, \
                    tc.tile_pool(name="inips", bufs=1, space="PSUM") as inips:
                self._damp_cs(ini, inips)

            for l in range(L):
                last = l == L - 1
                with tc.tile_pool(name=f"lay{l}", bufs=1) as layp, \
                        tc.tile_pool(name=f"wk{l}", bufs=1) as wk, \
                        tc.tile_pool(name=f"ps{l}", bufs=1, space="PSUM") as psp:
                    self.psp = psp
                    lay = self._layer_consts(l, layp)
                    for s in range(S):
                        self._sample(l, s, lay, wk)
                    if last:
                        for s in range(S):
                            self._output(s, wk)

        nc.compile()
        return nc

    # ---------- dampening cumsum -> csd [128, ND*HOR] ----------
    def _damp_cs(self, ini, inips):
        nc = self.nc
        ones = self.ones
        dcol = ini.tile([HEADS, 1], F32, name="dcol")
        nc.sync.dma_start(dcol[:], self.d_damp[:])
        df = ini.tile([HEADS, 1], F32, name="dfsig")
        nc.scalar.activation(df[:], dcol[:], AF.Sigmoid)
        dfb = ini.tile([HEADS, HOR], F32, name="dfb")
        nc.scalar.activation(dfb[:], ones[0:HEADS, 0:HOR], AF.Identity,
                             scale=df[:, 0:1])
        zer = ini.tile([HEADS, HOR], F32, name="zer8")
        nc.vector.memset(zer[:], 0.0)
        dfp = ini.tile([HEADS, HOR], F32, name="dfp")
        nc.vector.tensor_tensor_scan(dfp[:], dfb[:], zer[:], 1.0,
                                     op0=ALU.mult, op1=ALU.add)
        cs8 = ini.tile([HEADS, HOR], F32, name="cs8")
        nc.vector.tensor_tensor_scan(cs8[:], ones[0:HEADS, 0:HOR], dfp[:], 0.0,
                                     op0=ALU.mult, op1=ALU.add)
        for dt in range(ND):
            pini = inips.tile([128, HOR], F32, tag="pini", name="pini")
            nc.tensor.matmul(pini[:], self.e8t[:, _sl(dt)], cs8[:],
                             start=True, stop=True)
            nc.scalar.copy(self.csdt[:, dt * HOR:(dt + 1) * HOR], pini[:])

    # ---------- per-layer constants ----------
    def _layer_consts(self, l, layp):
        nc = self.nc
        ones = self.ones
        last = l == L - 1
        lay = {"l": l, "last": last}

        wdt = F32R if last else F32
        win = [layp.tile([128, D], wdt, name=f"win{k}") for k in range(ND)]
        wout = [layp.tile([128, D], wdt, name=f"wout{k}") for k in range(ND)]
        if last:
            for kt in range(ND):
                wtmp = layp.tile([128, D], F32, tag="wtmp", name="wtmp")
                nc.sync.dma_start(wtmp[:], self.d_win[l, _sl(kt), :])
                nc.vector.tensor_copy(win[kt][:], wtmp[:])
                wtmp2 = layp.tile([128, D], F32, tag="wtmp", name="wtmp")
                nc.sync.dma_start(wtmp2[:], self.d_wout[l, _sl(kt), :])
                nc.vector.tensor_copy(wout[kt][:], wtmp2[:])
        else:
            for kt in range(ND):
                nc.sync.dma_start(win[kt][:], self.d_win[l, _sl(kt), :])
                nc.sync.dma_start(wout[kt][:], self.d_wout[l, _sl(kt), :])

        # lrows: p0 = bout[512]; p32 = bg[7] then bp at cols 16..23
        lrows = layp.tile([128, 512], F32, name="lrows")
        nc.sync.dma_start(lrows[0:1, 0:D], self.d_bout[l, :, :])
        nc.sync.dma_start(lrows[32:33, 0:TF], self.d_bg[l, :, :])
        nc.sync.dma_start(lrows[32:33, 16:16 + TF], self.d_bp[l, :, :])

        # lcol pack [128, 16]: al(4) oma(4) init(4) bi(4); plus lv cols [7,1]
        lcol = layp.tile([128, 24], F32, name="lcol")
        al8 = layp.tile([HEADS, 1], F32, tag="al8t", name="al8")
        nc.sync.dma_start(al8[:], self.d_al8[l, :, :])
        al8s = layp.tile([HEADS, 1], F32, tag="al8s", name="al8s")
        nc.scalar.activation(al8s[:], al8[:], AF.Sigmoid)
        for dt in range(ND):
            pal = self.psp.tile([128, 1], F32, tag="bk0", name="pal")
            nc.tensor.matmul(pal[:], self.e8t[:, _sl(dt)], al8s[:],
                             start=True, stop=True)
            nc.scalar.copy(lcol[:, dt:dt + 1], pal[:])
        for dt in range(ND):
            nc.vector.tensor_scalar(lcol[:, 4 + dt:5 + dt], lcol[:, dt:dt + 1],
                                    -1.0, 1.0, op0=ALU.mult, op1=ALU.add)
            nc.sync.dma_start(lcol[:, 8 + dt:9 + dt],
                              self.d_init[l, _sl(dt)].rearrange("(a b) -> a b", b=1))
        bi = layp.tile([128, ND], F32, tag="bitmp", name="bitmp")
        for dt in range(ND):
            nc.sync.dma_start(bi[:, dt:dt + 1],
                              self.d_bin[l, _sl(dt)].rearrange("(a b) -> a b", b=1))
        nc.vector.tensor_sub(lcol[:, 12:16], bi[:], lcol[:, 8:12])
        # level alpha
        alv = layp.tile([1, 1], F32, tag="alvt", name="alv")
        nc.sync.dma_start(alv[:], self.d_alv[l, :, :])
        alvs = layp.tile([1, 1], F32, tag="alvst", name="alvs")
        nc.scalar.activation(alvs[:], alv[:], AF.Sigmoid)
        pv = self.psp.tile([TF, 1], F32, tag="bk1", name="palv")
        nc.tensor.matmul(pv[:], ones[0:1, 0:TF], alvs[:], start=True, stop=True)
        nc.scalar.copy(lcol[0:TF, 16:17], pv[:])
        nc.vector.tensor_scalar(lcol[0:TF, 17:18], lcol[0:TF, 16:17], -1.0, 1.0,
                                op0=ALU.mult, op1=ALU.add)

        # level weights [128, TF] x4 packed [128, 2*ND*TF], as fp32r
        lwf = layp.tile([128, 2 * ND * TF], F32, tag="lwf", name="lwf")
        for kt in range(ND):
            nc.sync.dma_start(lwf[:, kt * TF:(kt + 1) * TF], self.d_wg[l, _sl(kt), :])
            nc.sync.dma_start(lwf[:, (ND + kt) * TF:(ND + kt + 1) * TF],
                              self.d_wp[l, _sl(kt), :])
        lw = layp.tile([128, 2 * ND * TF], F32R, name="lw")
        nc.vector.tensor_copy(lw[:], lwf[:])

        lay.update(win=win, wout=wout, lrows=lrows, lcol=lcol, lw=lw)
        return lay

    # ---------- one sample through one layer ----------
    def _sample(self, l, s, lay, wk):
        nc = self.nc
        ones, idn = self.ones, self.idn
        last = lay["last"]
        agg = self.aggt

        def aggsl(dt):
            return self.aggt[:, (s * ND + dt) * HOR:(s * ND + dt + 1) * HOR]

        # --- z input: conv (l0) or reload (l1)
        z = [wk.tile([128, D], F32, tag=f"B1_{tt}", name=f"z{tt}")
             for tt in range(NT)]
        if l == 0:
            xsh = wk.tile([96, N], F32, tag="xsh", name="xsh")
            xts = wk.tile([TF, N], F32, tag="xts", name="xts")
            nc.sync.dma_start(xts[:], self.d_xT[s * TF:(s + 1) * TF, :])
            nc.vector.memset(xsh[:], 0.0)
            nc.vector.tensor_copy(xsh[0:TF, 1:N], xts[:, 0:N - 1])
            nc.vector.tensor_copy(xsh[32:32 + TF, 0:N], xts[:, 0:N])
            nc.vector.tensor_copy(xsh[64:64 + TF, 0:N - 1], xts[:, 1:N])
            for tt in range(NT):
                pz = self.bank(tt % 2)
                nc.tensor.matmul(pz[:], xsh[:, _sl(tt)], self.w2dt_[:],
                                 start=True, stop=False)
                nc.tensor.matmul(pz[:], ones[32:33, 0:128], self.rows[32:33, 0:512],
                                 start=False, stop=True)
                nc.scalar.copy(z[tt][:], pz[:])
        else:
            for tt in range(NT):
                nc.sync.dma_start(z[tt][:], self.zmid[s, _sl(tt), :])

        # --- rfft (fp32, dft streamed, 8 psum banks)
        psA = [self.bank(ct) for ct in range(ND)]
        psB = [self.bank(4 + ct) for ct in range(ND)]
        for kt in range(NT):
            dftk = wk.tile([128, 1024], F32, tag=f"dftk{kt % 2}", name="dftk")
            nc.sync.dma_start(dftk[:], self.d_dft[_sl(kt), :])
            for ct in range(ND):
                nc.tensor.matmul(psA[ct][:], z[kt][:, _sl(ct)], dftk[:, 0:512],
                                 start=(kt == 0), stop=(kt == NT - 1))
                nc.tensor.matmul(psB[ct][:], z[kt][:, _sl(ct)], dftk[:, 512:1024],
                                 start=(kt == 0), stop=(kt == NT - 1))

        # --- top-4 mask -> filt [ND][128, 1024] ([c, f])
        filt = [wk.tile([128, 1024], F32, tag=f"A1_{ct}", name=f"filt{ct}")
                for ct in range(ND)]
        for ct in range(ND):
            sqA = wk.tile([128, 512], F32, tag="sqA", name="sqA")
            nc.scalar.activation(sqA[:], psA[ct][:], AF.Square)
            sqB = wk.tile([128, 512], F32, tag="sqB", name="sqB")
            nc.scalar.activation(sqB[:], psB[ct][:], AF.Square)
            amp2 = wk.tile([128, 513], F32, tag="amp2", name="amp2")
            nc.vector.tensor_add(amp2[:, 1:512], sqA[:, 1:512], sqB[:, 1:512])
            nc.scalar.copy(amp2[:, 0:1], sqA[:, 0:1])
            nc.scalar.copy(amp2[:, 512:513], sqB[:, 0:1])
            top8 = wk.tile([128, 8], F32, tag="top8", name="top8")
            nc.vector.max(top8[:], amp2[:])
            kth = top8[:, 3:4]
            nc.vector.scalar_tensor_tensor(filt[ct][:, 0:512], amp2[:, 0:512],
                                           kth, psA[ct][:],
                                           op0=ALU.is_ge, op1=ALU.mult)
            nc.vector.scalar_tensor_tensor(filt[ct][:, 513:1024], amp2[:, 1:512],
                                           kth, psB[ct][:, 1:512],
                                           op0=ALU.is_ge, op1=ALU.mult)
            nc.vector.scalar_tensor_tensor(filt[ct][:, 512:513], amp2[:, 512:513],
                                           kth, psB[ct][:, 0:1],
                                           op0=ALU.is_ge, op1=ALU.mult)

        # --- transpose filt -> filtT [NT][128, 512] ([f, c])
        fdt = F32R if last else F32
        filtT = [wk.tile([128, 512], fdt, tag=f"B2_{ft}", name=f"filtT{ft}")
                 for ft in range(NT)]
        for ft in range(NT):
            pT = self.bank(ft % 2)
            for ct in range(ND):
                nc.tensor.transpose(pT[:, _sl(ct)], filt[ct][:, _sl(ft)], idn[:])
            if last:
                nc.vector.tensor_copy(filtT[ft][:], pT[:])
            else:
                nc.scalar.copy(filtT[ft][:], pT[:])

        # --- irfft (ib streamed, 8 banks) -> lp, z2
        pl = [self.bank(tt) for tt in range(NT)]
        for ft in range(NT):
            ibk = wk.tile([128, 1024], fdt, tag=f"ibk{ft % 2}", name="ibk")
            if last:
                ibf = wk.tile([128, 1024], F32, tag="ibf", name="ibf")
                nc.sync.dma_start(ibf[:], self.d_ib[_sl(ft), :])
                nc.vector.tensor_copy(ibk[:], ibf[:])
            else:
                nc.sync.dma_start(ibk[:], self.d_ib[_sl(ft), :])
            for tt in range(NT):
                nc.tensor.matmul(pl[tt][:], ibk[:, _sl(tt)], filtT[ft][:],
                                 start=(ft == 0), stop=(ft == NT - 1))
        lp = [wk.tile([128, D], F32, tag=f"B3_{tt}", name=f"lp{tt}")
              for tt in range(NT)]
        z2 = [wk.tile([128, D], F32, tag=f"B4_{tt}", name=f"z2_{tt}")
              for tt in range(NT)]
        for tt in range(NT):
            nc.scalar.copy(lp[tt][:], pl[tt][:])
            nc.vector.tensor_sub(z2[tt][:], z[tt][:], pl[tt][:])

        # --- lpT [ND][128, N] (tag A2) + extrap + perT; then free
        lpT = [wk.tile([128, N], F32R, tag=f"A2_{dt}", name=f"lpT{dt}")
               for dt in range(ND)]
        for dt in range(ND):
            for h in range(2):
                pT = self.bank(dt % 2)
                for q in range(4):
                    nc.tensor.transpose(pT[:, _sl(q)], lp[h * 4 + q][:, _sl(dt)],
                                        idn[:])
                nc.scalar.copy(lpT[dt][:, _hh(h)], pT[:])
            nc.vector.tensor_add(aggsl(dt), aggsl(dt), lpT[dt][:, 0:HOR])
        perT = wk.tile([TF, N], F32, tag="perT", name="perT")
        for h in range(2):
            pp = self.bank(2)
            for kt in range(ND):
                nc.tensor.matmul(pp[0:TF, :], lay["lw"][:, (ND + kt) * TF:(ND + kt + 1) * TF],
                                 lpT[kt][:, _hh(h)], start=(kt == 0), stop=False)
            nc.tensor.matmul(pp[0:TF, :], lay["lrows"][32:33, 16:16 + TF],
                             ones[32:33, 0:512], start=False, stop=True)
            nc.scalar.copy(perT[:, _hh(h)], pp[0:TF, :])

        # --- z2T (tag A2 reuse after lpT dead)
        zdt = F32R if last else F32
        z2T = [wk.tile([128, N], zdt, tag=f"A2_{dt}", name=f"z2T{dt}")
               for dt in range(ND)]
        for dt in range(ND):
            for h in range(2):
                pT = self.bank(dt % 2)
                for q in range(4):
                    nc.tensor.transpose(pT[:, _sl(q)], z2[h * 4 + q][:, _sl(dt)],
                                        idn[:])
                if last:
                    nc.vector.tensor_copy(z2T[dt][:, _hh(h)], pT[:])
                else:
                    nc.scalar.copy(z2T[dt][:, _hh(h)], pT[:])

        # --- win GEMM -> xinT (tag A1 reuse: filt dead)
        xinT = [wk.tile([128, N], F32, tag=f"A1_{dt}", name=f"xinT{dt}")
                for dt in range(ND)]
        for dt in range(ND):
            for h in range(2):
                px = self.bank(4 + dt % 2)
                for kt in range(ND):
                    nc.tensor.matmul(px[:], lay["win"][kt][:, _sl(dt)],
                                     z2T[kt][:, _hh(h)],
                                     start=(kt == 0), stop=(kt == ND - 1))
                nc.scalar.copy(xinT[dt][:, _hh(h)], px[:])

        # --- xd -> scan -> sT (tag A2 reuse: z2T dead)
        sdt = F32R if last else F32
        sT = [wk.tile([128, N], sdt, tag=f"A2_{dt}", name=f"sT{dt}")
              for dt in range(ND)]
        lc = lay["lcol"]
        for dt in range(ND):
            xd = wk.tile([128, N], F32, tag="xd", name="xd")
            nc.vector.tensor_sub(xd[:, 1:N], xinT[dt][:, 1:N], xinT[dt][:, 0:N - 1])
            nc.vector.tensor_scalar_add(xd[:, 0:1], xinT[dt][:, 0:1],
                                        lc[:, 12 + dt:13 + dt])
            nc.vector.tensor_scalar_mul(xd[:], xd[:], lc[:, dt:dt + 1])
            if OMA_BCAST:
                omab_ap = lc[:, 4 + dt:5 + dt].broadcast_to([128, N])
            else:
                omab = wk.tile([128, N], F32, tag="omab", name="omab")
                nc.vector.memset(omab[:], 1.0)
                nc.vector.tensor_scalar_mul(omab[:], omab[:], lc[:, 4 + dt:5 + dt])
                omab_ap = omab[:]
            nc.vector.tensor_tensor_scan(sT[dt][:], omab_ap, xd[:],
                                         lc[:, 8 + dt:9 + dt],
                                         op0=ALU.mult, op1=ALU.add)

        # --- wout GEMM -> lg [t,d] (tag B2 reuse: filtT dead) (+ z3 if l0)
        lg = [wk.tile([128, D], F32, tag=f"B2_{tt}", name=f"lg{tt}")
              for tt in range(NT)]
        for tt in range(NT):
            pg = self.bank(tt % 2)
            for kt in range(ND):
                nc.tensor.matmul(pg[:], sT[kt][:, _sl(tt)], lay["wout"][kt][:],
                                 start=(kt == 0), stop=False)
            nc.tensor.matmul(pg[:], ones[0:1, 0:128], lay["lrows"][0:1, 0:D],
                             start=False, stop=True)
            nc.scalar.copy(lg[tt][:], pg[:])
            if not last:
                # z3 overwrites z (tag B1): z dead after z2
                nc.vector.tensor_sub(z[tt][:], z2[tt][:], pg[:])
        z3 = z

        # --- lgT via transposes (tag A1 reuse: xinT dead)
        lgT = [wk.tile([128, N], F32R, tag=f"A1_{dt}", name=f"lgT{dt}")
               for dt in range(ND)]
        for dt in range(ND):
            for h in range(2):
                pT = self.bank(2 + dt % 2)
                for q in range(4):
                    nc.tensor.transpose(pT[:, _sl(q)], lg[h * 4 + q][:, _sl(dt)],
                                        idn[:])
                nc.scalar.copy(lgT[dt][:, _hh(h)], pT[:])
            # damp: agg += lg_last * csd
            nc.vector.scalar_tensor_tensor(
                aggsl(dt), self.csdt[:, dt * HOR:(dt + 1) * HOR],
                lgT[dt][:, N - 1:N], aggsl(dt), op0=ALU.mult, op1=ALU.add)

        # --- level: grT; scans update xtmid
        grT = wk.tile([TF, N], F32, tag="grT", name="grT")
        for h in range(2):
            pgr = self.bank(6)
            for kt in range(ND):
                nc.tensor.matmul(pgr[0:TF, :], lay["lw"][:, kt * TF:(kt + 1) * TF],
                                 lgT[kt][:, _hh(h)], start=(kt == 0), stop=False)
            nc.tensor.matmul(pgr[0:TF, :], lay["lrows"][32:33, 0:TF],
                             ones[32:33, 0:512], start=False, stop=True)
            nc.scalar.copy(grT[:, _hh(h)], pgr[0:TF, :])

        xts2 = wk.tile([TF, N], F32, tag="xts", name="xts2")
        if l == 0:
            nc.sync.dma_start(xts2[:], self.d_xT[s * TF:(s + 1) * TF, :])
        else:
            nc.sync.dma_start(xts2[:], self.xtmid[s, :, :])
        v = wk.tile([TF, N], F32, tag="lvv", name="lvv")
        nc.vector.tensor_sub(v[:], xts2[:], perT[:])
        nc.vector.tensor_scalar_mul(v[:], v[:], lc[0:TF, 16:17])
        if OMA_BCAST:
            omlv_ap = lc[0:TF, 17:18].broadcast_to([TF, N])
        else:
            omlv = wk.tile([TF, N], F32, tag="omlv", name="omlv")
            nc.vector.memset(omlv[:], 1.0)
            nc.vector.tensor_scalar_mul(omlv[:], omlv[:], lc[0:TF, 17:18])
            omlv_ap = omlv[:]
        pt = wk.tile([TF, N], F32, tag="lvp", name="lvp")
        nc.vector.tensor_tensor_scan(pt[:], omlv_ap, v[:], 0.0,
                                     op0=ALU.mult, op1=ALU.add)
        gt = wk.tile([TF, N], F32, tag="lvv", name="lvg")
        nc.vector.tensor_tensor_scan(gt[:], omlv_ap, grT[:], 0.0,
                                     op0=ALU.mult, op1=ALU.add)
        xnew = wk.tile([TF, N], F32, tag="grT", name="xnew")
        nc.vector.tensor_add(xnew[:], pt[:], gt[:])
        nc.sync.dma_start(self.xtmid[s, :, :], xnew[:])

        # --- FF (layer 0 only), then spill z4
        if not last:
            z4 = self._ff(s, z3, wk)
            for tt in range(NT):
                nc.sync.dma_start(self.zmid[s, _sl(tt), :], z4[tt][:])

    # ---------- LN stats ----------
    def _ln_stats(self, zset, wk, tagp):
        nc = self.nc
        st = wk.tile([128, 8 * NT], F32, tag=f"st{tagp}", name=f"st{tagp}")
        mu8 = st[:, 0:NT]
        s28 = st[:, NT:2 * NT]
        scr = wk.tile([128, D], F32, tag="lnscr", name="lnscr")
        for tt in range(NT):
            nc.vector.tensor_reduce(st[:, tt:tt + 1], zset[tt][:],
                                    mybir.AxisListType.X, op=ALU.add)
            nc.scalar.activation(scr[:], zset[tt][:], AF.Square,
                                 accum_out=st[:, NT + tt:NT + tt + 1])
        mun = st[:, 2 * NT:3 * NT]
        nc.vector.tensor_scalar_mul(mun, mu8, 1.0 / D)
        ex2 = st[:, 3 * NT:4 * NT]
        nc.vector.tensor_scalar_mul(ex2, s28, 1.0 / D)
        musq = st[:, 4 * NT:5 * NT]
        nc.scalar.activation(musq, mun, AF.Square)
        var = st[:, 5 * NT:6 * NT]
        nc.vector.tensor_sub(var, ex2, musq)
        sd = st[:, 6 * NT:7 * NT]
        nc.scalar.activation(sd, var, AF.Sqrt, bias=self.epst[:, 0:1])
        rs = st[:, 7 * NT:8 * NT]
        nc.vector.reciprocal(rs, sd)
        nmurs = st[:, 4 * NT:5 * NT]  # overwrite musq slot
        nc.vector.tensor_mul(nmurs, mun, rs)
        nc.vector.tensor_scalar_mul(nmurs, nmurs, -1.0)
        return rs, nmurs

    # ---------- FF block ----------
    def _ff(self, s, z3, wk):
        nc = self.nc
        ones, idn = self.ones, self.idn
        rows, cpk = self.rows, self.cpk
        rs, nmurs = self._ln_stats(z3, wk, "pre")
        # h = (z3-mu)*rs, overwrite z3 tiles in place via scratch
        h_ = [wk.tile([128, D], F32, tag=f"B2_{tt}", name=f"h{tt}")
              for tt in range(NT)]
        for tt in range(NT):
            nc.scalar.activation(h_[tt][:], z3[tt][:], AF.Identity,
                                 scale=rs[:, tt:tt + 1], bias=nmurs[:, tt:tt + 1])
        hT = [wk.tile([128, N], F32, tag=f"A2_{dt}", name=f"hT{dt}")
              for dt in range(ND)]
        for dt in range(ND):
            for h in range(2):
                pT = self.bank(dt % 2)
                for q in range(4):
                    nc.tensor.transpose(pT[:, _sl(q)], h_[h * 4 + q][:, _sl(dt)],
                                        idn[:])
                nc.scalar.copy(hT[dt][:, _hh(h)], pT[:])
        znT = [wk.tile([128, N], F32, tag=f"A1_{dt}", name=f"znT{dt}")
               for dt in range(ND)]
        for dt in range(ND):
            nc.scalar.activation(znT[dt][:], hT[dt][:], AF.Identity,
                                 scale=cpk[:, dt:dt + 1], bias=cpk[:, 4 + dt:5 + dt])

        yT = [wk.tile([128, N], F32, tag=f"A2_{dt}", name=f"yT{dt}")
              for dt in range(ND)]
        for h in range(2):
            pzf = [self.bank(dt) for dt in range(ND)]
            for m in range(NM):
                w1m = wk.tile([128, ND * 128], F32, tag=f"w1m{m % 2}", name="w1m")
                for kt in range(ND):
                    nc.sync.dma_start(w1m[:, _sl(kt)], self.d_ffw1[_sl(kt), _sl(m)])
                ph = self.bank(4 + m % 2)
                for kt in range(ND):
                    nc.tensor.matmul(ph[:], w1m[:, _sl(kt)], znT[kt][:, _hh(h)],
                                     start=(kt == 0), stop=(kt == ND - 1))
                sig = wk.tile([128, 512], F32, tag=f"sig{m % 2}", name="sig")
                nc.scalar.activation(sig[:], ph[:], AF.Sigmoid,
                                     bias=cpk[:, 8 + m:9 + m])
                w2m = wk.tile([128, 512], F32, tag=f"w2m{m % 2}", name="w2m")
                nc.sync.dma_start(w2m[:], self.d_ffw2[_sl(m), :])
                for dt in range(ND):
                    nc.tensor.matmul(pzf[dt][:], w2m[:, _sl(dt)], sig[:],
                                     start=(m == 0), stop=(m == NM - 1))
            for dt in range(ND):
                nc.vector.scalar_tensor_tensor(yT[dt][:, _hh(h)], pzf[dt][:],
                                               cpk[:, 24 + dt:25 + dt],
                                               znT[dt][:, _hh(h)],
                                               op0=ALU.add, op1=ALU.add)

        y = [wk.tile([128, D], F32, tag=f"B4_{tt}", name=f"y{tt}")
             for tt in range(NT)]
        for tt in range(NT):
            pT = self.bank(6)
            for dt in range(ND):
                nc.tensor.transpose(pT[:, _sl(dt)], yT[dt][:, _sl(tt)], idn[:])
            nc.scalar.copy(y[tt][:], pT[:])

        rs2, nmurs2 = self._ln_stats(y, wk, "post")
        gb = wk.tile([128, D], F32, tag="gpb", name="gpb")
        pb = self.bank(0)
        nc.tensor.matmul(pb[:], ones[32:33, 0:128], rows[32:33, 512:1024],
                         start=True, stop=True)
        nc.scalar.copy(gb[:], pb[:])
        bb = wk.tile([128, D], F32, tag="bpb", name="bpb")
        pb2 = self.bank(1)
        nc.tensor.matmul(pb2[:], ones[32:33, 0:128], rows[32:33, 1024:1536],
                         start=True, stop=True)
        nc.scalar.copy(bb[:], pb2[:])
        z4 = [wk.tile([128, D], F32, tag=f"B1_{tt}", name=f"z4_{tt}")
              for tt in range(NT)]
        scr2 = wk.tile([128, D], F32, tag="lnscr", name="scr2")
        for tt in range(NT):
            nc.scalar.activation(scr2[:], y[tt][:], AF.Identity,
                                 scale=rs2[:, tt:tt + 1], bias=nmurs2[:, tt:tt + 1])
            nc.vector.tensor_mul(z4[tt][:], scr2[:], gb[:])
            nc.vector.tensor_add(z4[tt][:], z4[tt][:], bb[:])
        return z4

    # ---------- output head ----------
    def _output(self, s, wk):
        nc = self.nc
        ones = self.ones
        po = self.bank(7)
        for kt in range(ND):
            nc.tensor.matmul(po[0:TF, 0:HOR], self.outwt[:, kt * TF:(kt + 1) * TF],
                             self.aggt[:, (s * ND + kt) * HOR:(s * ND + kt + 1) * HOR],
                             start=(kt == 0), stop=False)
        nc.tensor.matmul(po[0:TF, 0:HOR], self.rows[64:65, 0:TF],
                         ones[64:65, 0:HOR], start=False, stop=True)
        xfin = wk.tile([TF, N], F32, tag="xts", name="xfin")
        nc.sync.dma_start(xfin[:], self.xtmid[s, :, :])
        oT = wk.tile([TF, HOR], F32, tag="oT", name="oT")
        nc.vector.tensor_scalar_add(oT[:], po[0:TF, 0:HOR], xfin[:, N - 1:N])
        nc.sync.dma_start(self.d_out[s * TF:(s + 1) * TF, :], oT[:])


def _get_nc():
    if "nc" not in _CACHE:
        _CACHE["nc"] = K().build()
    return _CACHE["nc"]


def _common_maps(inputs, w2d, dft, ib, e8):
    return dict(
        w2d=w2d,
        convb=np.asarray(inputs["conv_b"], np.float32).reshape(1, D),
        dft=dft, ib=ib,
        idn=np.eye(128, dtype=np.float32),
        e8=e8,
        win=np.asarray(inputs["mhesa_win"], np.float32),
        wout=np.asarray(inputs["mhesa_wout"], np.float32),
        binr=np.asarray(inputs["mhesa_bin"], np.float32),
        boutr=np.asarray(inputs["mhesa_bout"], np.float32).reshape(L, 1, D),
        initf=np.asarray(inputs["mhesa_init"], np.float32).reshape(L, D),
        alpha8=np.asarray(inputs["mhesa_alpha"], np.float32).reshape(L, HEADS, 1),
        ffw1=np.asarray(inputs["ff_w1"], np.float32),
        ffb1=np.asarray(inputs["ff_b1"], np.float32).reshape(1, FD),
        ffw2=np.asarray(inputs["ff_w2"], np.float32),
        ffb2=np.asarray(inputs["ff_b2"], np.float32).reshape(1, D),
        gprec=np.asarray(inputs["ff_pre_g"], np.float32).reshape(D, 1),
        bprec=np.asarray(inputs["ff_pre_b"], np.float32).reshape(D, 1),
        gpostr=np.asarray(inputs["ff_post_g"], np.float32).reshape(1, D),
        bpostr=np.asarray(inputs["ff_post_b"], np.float32).reshape(1, D),
        lvwg=np.asarray(inputs["level_wg"], np.float32),
        lvwp=np.asarray(inputs["level_wp"], np.float32),
        lvbg=np.asarray(inputs["level_bg"], np.float32).reshape(L, 1, TF),
        lvbp=np.asarray(inputs["level_bp"], np.float32).reshape(L, 1, TF),
        lvalpha=np.asarray(inputs["level_alpha"], np.float32).reshape(L, 1, 1),
        damp8=np.asarray(inputs["dampen_factor"], np.float32).reshape(HEADS, 1),
        outw=np.asarray(inputs["out_w"], np.float32),
        outbr=np.asarray(inputs["out_b"], np.float32).reshape(1, TF),
    )


def _kernel_impl(inputs, runner):
    x = np.asarray(inputs["x"], np.float32)
    assert (x.shape[0], x.shape[1], x.shape[2]) == (32, N, TF)
    assert int(inputs["forecast_horizon"]) == HOR
    dft, ib = _dft_consts()
    conv_w = np.asarray(inputs["conv_w"], np.float32)
    w2d = _build_w2d(conv_w)
    e8 = np.repeat(np.eye(HEADS, dtype=np.float32), DH, axis=1)
    nc = _get_nc()
    common = _common_maps(inputs, w2d, dft, ib, e8)
    in_maps = []
    for c in range(NCORES):
        xs = x[c * S:(c + 1) * S]
        xT = xs.transpose(0, 2, 1).reshape(S * TF, N).copy()
        in_maps.append(dict(common, xT=xT))
    res = runner(nc, in_maps)
    out = np.zeros((x.shape[0], HOR, TF), np.float32)
    for c in range(NCORES):
        oT = res.results[c]["outT"].reshape(S, TF, HOR)
        out[c * S:(c + 1) * S] = oT.transpose(0, 2, 1)
    return out, res


def kernel(**inputs):
    out, _ = _kernel_impl(
        inputs,
        lambda nc, im: run_bass_kernel_spmd(nc, im, list(range(NCORES))))
    return out


def kernel_traced(**inputs):
    """Like kernel() but with NTFF profiling; returns (out, BassKernelResults)."""
    return _kernel_impl(
        inputs,
        lambda nc, im: run_bass_kernel_spmd(nc, im, list(range(NCORES)),
                                            trace=True))


# revision 16
# speedup vs baseline: 30.2203x; 1.0412x over previous
"""ETSFormer forward pass on 8 Trainium2 NeuronCores (Bass/Tile).

Data-parallel over batch: 32 samples -> 8 cores x 4 samples, weights
replicated, no collectives. The reference's FFT machinery is computed
exactly without FFTs:
  - freq_attention: dense DFT matmuls + hardware top-8 (vector.max) mask
  - mhesa / level exponential smoothing: the reference FFT cross-correlation
    is exactly a first-order EMA -> hardware prefix scan (tensor_tensor_scan)
  - fourier_extrapolate: exact slice (Dirichlet kernel identity)

Precision: the top-4 frequency mask is extremely sensitive (2e-4 relative
amp noise -> 2.6e-2 output error), so every GEMM feeding a ranking (conv,
rfft both layers, irfft/mhesa/FF of layer 0) runs in fp32; post-ranking
paths (layer-1 irfft/mhesa via lgT, level, damp, output head) run fp32r.
"""
import numpy as np
from contextlib import ExitStack

import concourse.bass as bass
import concourse.bacc as bacc
import concourse.tile as tile
from concourse import mybir
from concourse.bass_utils import run_bass_kernel_spmd

F32 = mybir.dt.float32
F32R = mybir.dt.float32r
BF16 = mybir.dt.bfloat16
AF = mybir.ActivationFunctionType
ALU = mybir.AluOpType

N = 1024
D = 512
TF = 7
HEADS = 8
DH = D // HEADS
L = 2
S = 4
NCORES = 8
HOR = 96
FD = 2048
NT = N // 128   # 8
ND = D // 128   # 4
NM = FD // 128  # 16

_CACHE = {}
OMA_BCAST = True


def _dft_consts():
    if "dft" not in _CACHE:
        t = np.arange(N)
        f = np.arange(513)
        ang = 2.0 * np.pi * np.outer(t, f) / N
        cos = np.cos(ang)
        sin = np.sin(ang)
        dft = np.zeros((N, 1024), np.float64)
        dft[:, 0:512] = cos[:, 0:512]
        dft[:, 512] = cos[:, 512]
        dft[:, 513:1024] = sin[:, 1:512]
        c = np.full(513, 2.0)
        c[0] = 1.0
        c[512] = 1.0
        ib = np.zeros((1024, N), np.float64)
        ib[0:512, :] = (c[0:512, None] / N) * cos[:, 0:512].T
        ib[512, :] = (1.0 / N) * cos[:, 512]
        ib[513:1024, :] = (2.0 / N) * sin[:, 1:512].T
        _CACHE["dft"] = dft.astype(np.float32)
        _CACHE["ib"] = ib.astype(np.float32)
    return _CACHE["dft"], _CACHE["ib"]


def _sl(i, w=128):
    return slice(i * w, (i + 1) * w)


def _split_hi(x):
    import ml_dtypes
    return x.astype(ml_dtypes.bfloat16)


def _split_lo(x):
    import ml_dtypes
    hi = x.astype(ml_dtypes.bfloat16).astype(np.float32)
    return (x - hi).astype(ml_dtypes.bfloat16)


def _build_w2d(conv_w):
    w2d = np.zeros((96, D), np.float32)
    for k in range(3):
        for c in range(TF):
            w2d[32 * k + c] = conv_w[:, c, k]
    return w2d


def _hh(h):
    return slice(h * 512, (h + 1) * 512)


class K:
    def __init__(self):
        nc = bacc.Bacc()
        self.nc = nc
        p = nc.declare_dram_parameter
        self.d_xT = p("xT", [S * TF, N], F32, isOutput=False)
        self.d_w2d = p("w2d", [96, D], F32, isOutput=False)
        self.d_dft = p("dft", [N, 1024], F32, isOutput=False)
        self.d_ib = p("ib", [1024, N], F32, isOutput=False)
        self.d_idn = p("idn", [128, 128], F32, isOutput=False)
        self.d_e8 = p("e8", [HEADS, D], F32, isOutput=False)
        self.d_win = p("win", [L, D, D], F32, isOutput=False)
        self.d_wout = p("wout", [L, D, D], F32, isOutput=False)
        self.d_bin = p("binr", [L, D], F32, isOutput=False)
        self.d_bout = p("boutr", [L, 1, D], F32, isOutput=False)
        self.d_init = p("initf", [L, D], F32, isOutput=False)
        self.d_al8 = p("alpha8", [L, HEADS, 1], F32, isOutput=False)
        self.d_ffw1 = p("ffw1", [D, FD], F32, isOutput=False)
        self.d_ffw1h = p("ffw1h", [D, FD], BF16, isOutput=False)
        self.d_ffw1l = p("ffw1l", [D, FD], BF16, isOutput=False)
        self.d_ffb1 = p("ffb1", [FD, 1], F32, isOutput=False)
        self.d_ffw2 = p("ffw2", [FD, D], F32, isOutput=False)
        self.d_ffb2 = p("ffb2", [D, 1], F32, isOutput=False)
        self.d_convb = p("convb", [1, D], F32, isOutput=False)
        self.d_gpre = p("gprec", [D, 1], F32, isOutput=False)
        self.d_bpre = p("bprec", [D, 1], F32, isOutput=False)
        self.d_gpost = p("gpostr", [1, D], F32, isOutput=False)
        self.d_bpost = p("bpostr", [1, D], F32, isOutput=False)
        self.d_wg = p("lvwg", [L, D, TF], F32, isOutput=False)
        self.d_wp = p("lvwp", [L, D, TF], F32, isOutput=False)
        self.d_bg = p("lvbg", [L, 1, TF], F32, isOutput=False)
        self.d_bp = p("lvbp", [L, 1, TF], F32, isOutput=False)
        self.d_alv = p("lvalpha", [L, 1, 1], F32, isOutput=False)
        self.d_damp = p("damp8", [HEADS, 1], F32, isOutput=False)
        self.d_outw = p("outw", [D, TF], F32, isOutput=False)
        self.d_outb = p("outbr", [1, TF], F32, isOutput=False)
        self.d_out = p("outT", [S * TF, HOR], F32, isOutput=True)
        self.zmid = nc.dram_tensor("zmid", [S, N, D], F32)
        self.xtmid = nc.dram_tensor("xtmid", [S, TF, N], F32)

    # psum bank helper: tag-based reuse of the 8 banks
    def bank(self, i, shape=(128, 512)):
        tl = self.psp.tile(list(shape), F32, tag=f"bk{i}", name=f"bk{i}")
        return tl

    def build(self):
        nc = self.nc
        with ExitStack() as ctx:
            self.tc = ctx.enter_context(tile.TileContext(nc))
            tc = self.tc
            top = ctx.enter_context(tc.tile_pool(name="top", bufs=1))

            idn = top.tile([128, 128], F32, name="idn")
            nc.sync.dma_start(idn[:], self.d_idn[:])
            ones = top.tile([128, N], F32, name="ones")
            nc.vector.memset(ones[:], 1.0)
            e8 = top.tile([HEADS, D], F32, name="e8")
            nc.sync.dma_start(e8[:], self.d_e8[:])
            w2d = top.tile([96, D], F32, name="w2d")
            nc.sync.dma_start(w2d[:], self.d_w2d[:])
            # rows pack: p0 = ffb1[2048]; p32 = convb|gpost|bpost|ffb2 (4x512);
            # p64 = outb[7]
            rows = top.tile([128, FD], F32, name="rows")
            nc.sync.dma_start(rows[32:33, 0:512], self.d_convb[:])
            nc.sync.dma_start(rows[32:33, 512:1024], self.d_gpost[:])
            nc.sync.dma_start(rows[32:33, 1024:1536], self.d_bpost[:])
            nc.sync.dma_start(rows[64:65, 0:TF], self.d_outb[:])
            # col pack: gpre(4) | bpre(4)
            cpk = top.tile([128, 28], F32, name="cpk")
            for dt in range(ND):
                nc.sync.dma_start(cpk[:, dt:dt + 1], self.d_gpre[_sl(dt), :])
                nc.sync.dma_start(cpk[:, 4 + dt:5 + dt], self.d_bpre[_sl(dt), :])
            for m in range(NM):
                nc.sync.dma_start(cpk[:, 8 + m:9 + m], self.d_ffb1[_sl(m), :])
            for dt in range(ND):
                nc.sync.dma_start(cpk[:, 24 + dt:25 + dt],
                                  self.d_ffb2[_sl(dt), :])
            outw = top.tile([128, ND * TF], F32, name="outw")
            for kt in range(ND):
                nc.sync.dma_start(outw[:, kt * TF:(kt + 1) * TF],
                                  self.d_outw[_sl(kt), :])
            eps = top.tile([128, 1], F32, name="eps")
            nc.vector.memset(eps[:], 1e-5)
            self.epst = eps
            agg = top.tile([128, S * ND * HOR], F32, name="agg")
            nc.vector.memset(agg[:], 0.0)
            csd = top.tile([128, ND * HOR], F32, name="csd")

            self.idn, self.ones, self.rows, self.cpk = idn, ones, rows, cpk
            self.e8t, self.w2dt_, self.aggt, self.csdt = e8, w2d, agg, csd
            self.outwt = outw

            with tc.tile_pool(name="ini", bufs=1) as ini, \
                    tc.tile_pool(name="inips", bufs=1, space="PSUM") as inips:
                self._damp_cs(ini, inips)

            for l in range(L):
                last = l == L - 1
                with tc.tile_pool(name=f"lay{l}", bufs=1) as layp, \
                        tc.tile_pool(name=f"wk{l}", bufs=1) as wk, \
                        tc.tile_pool(name=f"ps{l}", bufs=1, space="PSUM") as psp:
                    self.psp = psp
                    lay = self._layer_consts(l, layp)
                    for s in range(S):
                        self._sample(l, s, lay, wk)
                    if last:
                        for s in range(S):
                            self._output(s, wk)

        nc.compile()
        return nc

    # ---------- dampening cumsum -> csd [128, ND*HOR] ----------
    def _damp_cs(self, ini, inips):
        nc = self.nc
        ones = self.ones
        dcol = ini.tile([HEADS, 1], F32, name="dcol")
        nc.sync.dma_start(dcol[:], self.d_damp[:])
        df = ini.tile([HEADS, 1], F32, name="dfsig")
        nc.scalar.activation(df[:], dcol[:], AF.Sigmoid)
        dfb = ini.tile([HEADS, HOR], F32, name="dfb")
        nc.scalar.activation(dfb[:], ones[0:HEADS, 0:HOR], AF.Identity,
                             scale=df[:, 0:1])
        zer = ini.tile([HEADS, HOR], F32, name="zer8")
        nc.vector.memset(zer[:], 0.0)
        dfp = ini.tile([HEADS, HOR], F32, name="dfp")
        nc.vector.tensor_tensor_scan(dfp[:], dfb[:], zer[:], 1.0,
                                     op0=ALU.mult, op1=ALU.add)
        cs8 = ini.tile([HEADS, HOR], F32, name="cs8")
        nc.vector.tensor_tensor_scan(cs8[:], ones[0:HEADS, 0:HOR], dfp[:], 0.0,
                                     op0=ALU.mult, op1=ALU.add)
        for dt in range(ND):
            pini = inips.tile([128, HOR], F32, tag="pini", name="pini")
            nc.tensor.matmul(pini[:], self.e8t[:, _sl(dt)], cs8[:],
                             start=True, stop=True)
            nc.scalar.copy(self.csdt[:, dt * HOR:(dt + 1) * HOR], pini[:])

    # ---------- per-layer constants ----------
    def _layer_consts(self, l, layp):
        nc = self.nc
        ones = self.ones
        last = l == L - 1
        lay = {"l": l, "last": last}

        wdt = F32R if last else F32
        win = [layp.tile([128, D], wdt, name=f"win{k}") for k in range(ND)]
        wout = [layp.tile([128, D], wdt, name=f"wout{k}") for k in range(ND)]
        if last:
            for kt in range(ND):
                wtmp = layp.tile([128, D], F32, tag="wtmp", name="wtmp")
                nc.sync.dma_start(wtmp[:], self.d_win[l, _sl(kt), :])
                nc.vector.tensor_copy(win[kt][:], wtmp[:])
                wtmp2 = layp.tile([128, D], F32, tag="wtmp", name="wtmp")
                nc.sync.dma_start(wtmp2[:], self.d_wout[l, _sl(kt), :])
                nc.vector.tensor_copy(wout[kt][:], wtmp2[:])
        else:
            for kt in range(ND):
                nc.sync.dma_start(win[kt][:], self.d_win[l, _sl(kt), :])
                nc.sync.dma_start(wout[kt][:], self.d_wout[l, _sl(kt), :])

        # lrows: p0 = bout[512]; p32 = bg[7] then bp at cols 16..23
        lrows = layp.tile([128, 512], F32, name="lrows")
        nc.sync.dma_start(lrows[0:1, 0:D], self.d_bout[l, :, :])
        nc.sync.dma_start(lrows[32:33, 0:TF], self.d_bg[l, :, :])
        nc.sync.dma_start(lrows[32:33, 16:16 + TF], self.d_bp[l, :, :])

        # lcol pack [128, 16]: al(4) oma(4) init(4) bi(4); plus lv cols [7,1]
        lcol = layp.tile([128, 24], F32, name="lcol")
        al8 = layp.tile([HEADS, 1], F32, tag="al8t", name="al8")
        nc.sync.dma_start(al8[:], self.d_al8[l, :, :])
        al8s = layp.tile([HEADS, 1], F32, tag="al8s", name="al8s")
        nc.scalar.activation(al8s[:], al8[:], AF.Sigmoid)
        for dt in range(ND):
            pal = self.psp.tile([128, 1], F32, tag="bk0", name="pal")
            nc.tensor.matmul(pal[:], self.e8t[:, _sl(dt)], al8s[:],
                             start=True, stop=True)
            nc.scalar.copy(lcol[:, dt:dt + 1], pal[:])
        for dt in range(ND):
            nc.vector.tensor_scalar(lcol[:, 4 + dt:5 + dt], lcol[:, dt:dt + 1],
                                    -1.0, 1.0, op0=ALU.mult, op1=ALU.add)
            nc.sync.dma_start(lcol[:, 8 + dt:9 + dt],
                              self.d_init[l, _sl(dt)].rearrange("(a b) -> a b", b=1))
        bi = layp.tile([128, ND], F32, tag="bitmp", name="bitmp")
        for dt in range(ND):
            nc.sync.dma_start(bi[:, dt:dt + 1],
                              self.d_bin[l, _sl(dt)].rearrange("(a b) -> a b", b=1))
        nc.vector.tensor_sub(lcol[:, 12:16], bi[:], lcol[:, 8:12])
        # level alpha
        alv = layp.tile([1, 1], F32, tag="alvt", name="alv")
        nc.sync.dma_start(alv[:], self.d_alv[l, :, :])
        alvs = layp.tile([1, 1], F32, tag="alvst", name="alvs")
        nc.scalar.activation(alvs[:], alv[:], AF.Sigmoid)
        pv = self.psp.tile([TF, 1], F32, tag="bk1", name="palv")
        nc.tensor.matmul(pv[:], ones[0:1, 0:TF], alvs[:], start=True, stop=True)
        nc.scalar.copy(lcol[0:TF, 16:17], pv[:])
        nc.vector.tensor_scalar(lcol[0:TF, 17:18], lcol[0:TF, 16:17], -1.0, 1.0,
                                op0=ALU.mult, op1=ALU.add)

        # level weights [128, TF] x4 packed [128, 2*ND*TF], as fp32r
        lwf = layp.tile([128, 2 * ND * TF], F32, tag="lwf", name="lwf")
        for kt in range(ND):
            nc.sync.dma_start(lwf[:, kt * TF:(kt + 1) * TF], self.d_wg[l, _sl(kt), :])
            nc.sync.dma_start(lwf[:, (ND + kt) * TF:(ND + kt + 1) * TF],
                              self.d_wp[l, _sl(kt), :])
        lw = layp.tile([128, 2 * ND * TF], F32R, name="lw")
        nc.vector.tensor_copy(lw[:], lwf[:])

        lay.update(win=win, wout=wout, lrows=lrows, lcol=lcol, lw=lw)
        return lay

    # ---------- one sample through one layer ----------
    def _sample(self, l, s, lay, wk):
        nc = self.nc
        ones, idn = self.ones, self.idn
        last = lay["last"]
        agg = self.aggt

        def aggsl(dt):
            return self.aggt[:, (s * ND + dt) * HOR:(s * ND + dt + 1) * HOR]

        # --- z input: conv (l0) or reload (l1)
        z = [wk.tile([128, D], F32, tag=f"B1_{tt}", name=f"z{tt}")
             for tt in range(NT)]
        if l == 0:
            xsh = wk.tile([96, N], F32, tag="xsh", name="xsh")
            xts = wk.tile([TF, N], F32, tag="xts", name="xts")
            nc.sync.dma_start(xts[:], self.d_xT[s * TF:(s + 1) * TF, :])
            nc.vector.memset(xsh[:], 0.0)
            nc.vector.tensor_copy(xsh[0:TF, 1:N], xts[:, 0:N - 1])
            nc.vector.tensor_copy(xsh[32:32 + TF, 0:N], xts[:, 0:N])
            nc.vector.tensor_copy(xsh[64:64 + TF, 0:N - 1], xts[:, 1:N])
            for tt in range(NT):
                pz = self.bank(tt % 2)
                nc.tensor.matmul(pz[:], xsh[:, _sl(tt)], self.w2dt_[:],
                                 start=True, stop=False)
                nc.tensor.matmul(pz[:], ones[32:33, 0:128], self.rows[32:33, 0:512],
                                 start=False, stop=True)
                nc.scalar.copy(z[tt][:], pz[:])
        else:
            for tt in range(NT):
                nc.sync.dma_start(z[tt][:], self.zmid[s, _sl(tt), :])

        # --- rfft (fp32, dft streamed, 8 psum banks)
        psA = [self.bank(ct) for ct in range(ND)]
        psB = [self.bank(4 + ct) for ct in range(ND)]
        for kt in range(NT):
            dftk = wk.tile([128, 1024], F32, tag=f"dftk{kt % 2}", name="dftk")
            nc.sync.dma_start(dftk[:], self.d_dft[_sl(kt), :])
            for ct in range(ND):
                nc.tensor.matmul(psA[ct][:], z[kt][:, _sl(ct)], dftk[:, 0:512],
                                 start=(kt == 0), stop=(kt == NT - 1))
                nc.tensor.matmul(psB[ct][:], z[kt][:, _sl(ct)], dftk[:, 512:1024],
                                 start=(kt == 0), stop=(kt == NT - 1))

        # --- top-4 mask -> filt [ND][128, 1024] ([c, f])
        filt = [wk.tile([128, 1024], F32, tag=f"A1_{ct}", name=f"filt{ct}")
                for ct in range(ND)]
        for ct in range(ND):
            sqA = wk.tile([128, 512], F32, tag="sqA", name="sqA")
            nc.scalar.activation(sqA[:], psA[ct][:], AF.Square)
            sqB = wk.tile([128, 512], F32, tag="sqB", name="sqB")
            nc.scalar.activation(sqB[:], psB[ct][:], AF.Square)
            amp2 = wk.tile([128, 513], F32, tag="amp2", name="amp2")
            nc.vector.tensor_add(amp2[:, 1:512], sqA[:, 1:512], sqB[:, 1:512])
            nc.scalar.copy(amp2[:, 0:1], sqA[:, 0:1])
            nc.scalar.copy(amp2[:, 512:513], sqB[:, 0:1])
            top8 = wk.tile([128, 8], F32, tag="top8", name="top8")
            nc.vector.max(top8[:], amp2[:])
            kth = top8[:, 3:4]
            nc.vector.scalar_tensor_tensor(filt[ct][:, 0:512], amp2[:, 0:512],
                                           kth, psA[ct][:],
                                           op0=ALU.is_ge, op1=ALU.mult)
            nc.vector.scalar_tensor_tensor(filt[ct][:, 513:1024], amp2[:, 1:512],
                                           kth, psB[ct][:, 1:512],
                                           op0=ALU.is_ge, op1=ALU.mult)
            nc.vector.scalar_tensor_tensor(filt[ct][:, 512:513], amp2[:, 512:513],
                                           kth, psB[ct][:, 0:1],
                                           op0=ALU.is_ge, op1=ALU.mult)

        # --- transpose filt -> filtT [NT][128, 512] ([f, c])
        fdt = F32R if last else F32
        filtT = [wk.tile([128, 512], fdt, tag=f"B2_{ft}", name=f"filtT{ft}")
                 for ft in range(NT)]
        for ft in range(NT):
            pT = self.bank(ft % 2)
            for ct in range(ND):
                nc.tensor.transpose(pT[:, _sl(ct)], filt[ct][:, _sl(ft)], idn[:])
            if last:
                nc.vector.tensor_copy(filtT[ft][:], pT[:])
            else:
                nc.scalar.copy(filtT[ft][:], pT[:])

        # --- irfft (ib streamed, 8 banks) -> lp, z2
        pl = [self.bank(tt) for tt in range(NT)]
        for ft in range(NT):
            ibk = wk.tile([128, 1024], fdt, tag=f"ibk{ft % 2}", name="ibk")
            if last:
                ibf = wk.tile([128, 1024], F32, tag="ibf", name="ibf")
                nc.sync.dma_start(ibf[:], self.d_ib[_sl(ft), :])
                nc.vector.tensor_copy(ibk[:], ibf[:])
            else:
                nc.sync.dma_start(ibk[:], self.d_ib[_sl(ft), :])
            for tt in range(NT):
                nc.tensor.matmul(pl[tt][:], ibk[:, _sl(tt)], filtT[ft][:],
                                 start=(ft == 0), stop=(ft == NT - 1))
        lp = [wk.tile([128, D], F32, tag=f"B3_{tt}", name=f"lp{tt}")
              for tt in range(NT)]
        z2 = [wk.tile([128, D], F32, tag=f"B4_{tt}", name=f"z2_{tt}")
              for tt in range(NT)]
        for tt in range(NT):
            nc.scalar.copy(lp[tt][:], pl[tt][:])
            nc.vector.tensor_sub(z2[tt][:], z[tt][:], pl[tt][:])

        # --- lpT [ND][128, N] (tag A2) + extrap + perT; then free
        lpT = [wk.tile([128, N], F32R, tag=f"A2_{dt}", name=f"lpT{dt}")
               for dt in range(ND)]
        for dt in range(ND):
            for h in range(2):
                pT = self.bank(dt % 2)
                for q in range(4):
                    nc.tensor.transpose(pT[:, _sl(q)], lp[h * 4 + q][:, _sl(dt)],
                                        idn[:])
                nc.scalar.copy(lpT[dt][:, _hh(h)], pT[:])
            nc.vector.tensor_add(aggsl(dt), aggsl(dt), lpT[dt][:, 0:HOR])
        perT = wk.tile([TF, N], F32, tag="perT", name="perT")
        for h in range(2):
            pp = self.bank(2)
            for kt in range(ND):
                nc.tensor.matmul(pp[0:TF, :], lay["lw"][:, (ND + kt) * TF:(ND + kt + 1) * TF],
                                 lpT[kt][:, _hh(h)], start=(kt == 0), stop=False)
            nc.tensor.matmul(pp[0:TF, :], lay["lrows"][32:33, 16:16 + TF],
                             ones[32:33, 0:512], start=False, stop=True)
            nc.scalar.copy(perT[:, _hh(h)], pp[0:TF, :])

        # --- z2T (tag A2 reuse after lpT dead)
        zdt = F32R if last else F32
        z2T = [wk.tile([128, N], zdt, tag=f"A2_{dt}", name=f"z2T{dt}")
               for dt in range(ND)]
        for dt in range(ND):
            for h in range(2):
                pT = self.bank(dt % 2)
                for q in range(4):
                    nc.tensor.transpose(pT[:, _sl(q)], z2[h * 4 + q][:, _sl(dt)],
                                        idn[:])
                if last:
                    nc.vector.tensor_copy(z2T[dt][:, _hh(h)], pT[:])
                else:
                    nc.scalar.copy(z2T[dt][:, _hh(h)], pT[:])

        # --- win GEMM -> xinT (tag A1 reuse: filt dead)
        xinT = [wk.tile([128, N], F32, tag=f"A1_{dt}", name=f"xinT{dt}")
                for dt in range(ND)]
        for dt in range(ND):
            for h in range(2):
                px = self.bank(4 + dt % 2)
                for kt in range(ND):
                    nc.tensor.matmul(px[:], lay["win"][kt][:, _sl(dt)],
                                     z2T[kt][:, _hh(h)],
                                     start=(kt == 0), stop=(kt == ND - 1))
                nc.scalar.copy(xinT[dt][:, _hh(h)], px[:])

        # --- xd -> scan -> sT (tag A2 reuse: z2T dead)
        sdt = F32R if last else F32
        sT = [wk.tile([128, N], sdt, tag=f"A2_{dt}", name=f"sT{dt}")
              for dt in range(ND)]
        lc = lay["lcol"]
        for dt in range(ND):
            xd = wk.tile([128, N], F32, tag="xd", name="xd")
            nc.vector.tensor_sub(xd[:, 1:N], xinT[dt][:, 1:N], xinT[dt][:, 0:N - 1])
            nc.vector.tensor_scalar_add(xd[:, 0:1], xinT[dt][:, 0:1],
                                        lc[:, 12 + dt:13 + dt])
            nc.vector.tensor_scalar_mul(xd[:], xd[:], lc[:, dt:dt + 1])
            if OMA_BCAST:
                omab_ap = lc[:, 4 + dt:5 + dt].broadcast_to([128, N])
            else:
                omab = wk.tile([128, N], F32, tag="omab", name="omab")
                nc.vector.memset(omab[:], 1.0)
                nc.vector.tensor_scalar_mul(omab[:], omab[:], lc[:, 4 + dt:5 + dt])
                omab_ap = omab[:]
            nc.vector.tensor_tensor_scan(sT[dt][:], omab_ap, xd[:],
                                         lc[:, 8 + dt:9 + dt],
                                         op0=ALU.mult, op1=ALU.add)

        # --- wout GEMM -> lg [t,d] (tag B2 reuse: filtT dead) (+ z3 if l0)
        lg = [wk.tile([128, D], F32, tag=f"B2_{tt}", name=f"lg{tt}")
              for tt in range(NT)]
        for tt in range(NT):
            pg = self.bank(tt % 2)
            for kt in range(ND):
                nc.tensor.matmul(pg[:], sT[kt][:, _sl(tt)], lay["wout"][kt][:],
                                 start=(kt == 0), stop=False)
            nc.tensor.matmul(pg[:], ones[0:1, 0:128], lay["lrows"][0:1, 0:D],
                             start=False, stop=True)
            nc.scalar.copy(lg[tt][:], pg[:])
            if not last:
                # z3 overwrites z (tag B1): z dead after z2
                nc.vector.tensor_sub(z[tt][:], z2[tt][:], pg[:])
        z3 = z

        # --- lgT via transposes (tag A1 reuse: xinT dead)
        lgT = [wk.tile([128, N], F32R, tag=f"A1_{dt}", name=f"lgT{dt}")
               for dt in range(ND)]
        for dt in range(ND):
            for h in range(2):
                pT = self.bank(2 + dt % 2)
                for q in range(4):
                    nc.tensor.transpose(pT[:, _sl(q)], lg[h * 4 + q][:, _sl(dt)],
                                        idn[:])
                nc.scalar.copy(lgT[dt][:, _hh(h)], pT[:])
            # damp: agg += lg_last * csd
            nc.vector.scalar_tensor_tensor(
                aggsl(dt), self.csdt[:, dt * HOR:(dt + 1) * HOR],
                lgT[dt][:, N - 1:N], aggsl(dt), op0=ALU.mult, op1=ALU.add)

        # --- level: grT; scans update xtmid
        grT = wk.tile([TF, N], F32, tag="grT", name="grT")
        for h in range(2):
            pgr = self.bank(6)
            for kt in range(ND):
                nc.tensor.matmul(pgr[0:TF, :], lay["lw"][:, kt * TF:(kt + 1) * TF],
                                 lgT[kt][:, _hh(h)], start=(kt == 0), stop=False)
            nc.tensor.matmul(pgr[0:TF, :], lay["lrows"][32:33, 0:TF],
                             ones[32:33, 0:512], start=False, stop=True)
            nc.scalar.copy(grT[:, _hh(h)], pgr[0:TF, :])

        xts2 = wk.tile([TF, N], F32, tag="xts", name="xts2")
        if l == 0:
            nc.sync.dma_start(xts2[:], self.d_xT[s * TF:(s + 1) * TF, :])
        else:
            nc.sync.dma_start(xts2[:], self.xtmid[s, :, :])
        v = wk.tile([TF, N], F32, tag="lvv", name="lvv")
        nc.vector.tensor_sub(v[:], xts2[:], perT[:])
        nc.vector.tensor_scalar_mul(v[:], v[:], lc[0:TF, 16:17])
        if OMA_BCAST:
            omlv_ap = lc[0:TF, 17:18].broadcast_to([TF, N])
        else:
            omlv = wk.tile([TF, N], F32, tag="omlv", name="omlv")
            nc.vector.memset(omlv[:], 1.0)
            nc.vector.tensor_scalar_mul(omlv[:], omlv[:], lc[0:TF, 17:18])
            omlv_ap = omlv[:]
        pt = wk.tile([TF, N], F32, tag="lvp", name="lvp")
        nc.vector.tensor_tensor_scan(pt[:], omlv_ap, v[:], 0.0,
                                     op0=ALU.mult, op1=ALU.add)
        gt = wk.tile([TF, N], F32, tag="lvv", name="lvg")
        nc.vector.tensor_tensor_scan(gt[:], omlv_ap, grT[:], 0.0,
                                     op0=ALU.mult, op1=ALU.add)
        xnew = wk.tile([TF, N], F32, tag="grT", name="xnew")
        nc.vector.tensor_add(xnew[:], pt[:], gt[:])
        nc.sync.dma_start(self.xtmid[s, :, :], xnew[:])

        # --- FF (layer 0 only), then spill z4
        if not last:
            z4 = self._ff(s, z3, wk)
            for tt in range(NT):
                nc.sync.dma_start(self.zmid[s, _sl(tt), :], z4[tt][:])

    # ---------- LN stats ----------
    def _ln_stats(self, zset, wk, tagp):
        nc = self.nc
        st = wk.tile([128, 8 * NT], F32, tag=f"st{tagp}", name=f"st{tagp}")
        mu8 = st[:, 0:NT]
        s28 = st[:, NT:2 * NT]
        scr = wk.tile([128, D], F32, tag="lnscr", name="lnscr")
        for tt in range(NT):
            nc.vector.tensor_reduce(st[:, tt:tt + 1], zset[tt][:],
                                    mybir.AxisListType.X, op=ALU.add)
            nc.scalar.activation(scr[:], zset[tt][:], AF.Square,
                                 accum_out=st[:, NT + tt:NT + tt + 1])
        mun = st[:, 2 * NT:3 * NT]
        nc.vector.tensor_scalar_mul(mun, mu8, 1.0 / D)
        ex2 = st[:, 3 * NT:4 * NT]
        nc.vector.tensor_scalar_mul(ex2, s28, 1.0 / D)
        musq = st[:, 4 * NT:5 * NT]
        nc.scalar.activation(musq, mun, AF.Square)
        var = st[:, 5 * NT:6 * NT]
        nc.vector.tensor_sub(var, ex2, musq)
        sd = st[:, 6 * NT:7 * NT]
        nc.scalar.activation(sd, var, AF.Sqrt, bias=self.epst[:, 0:1])
        rs = st[:, 7 * NT:8 * NT]
        nc.vector.reciprocal(rs, sd)
        nmurs = st[:, 4 * NT:5 * NT]  # overwrite musq slot
        nc.vector.tensor_mul(nmurs, mun, rs)
        nc.vector.tensor_scalar_mul(nmurs, nmurs, -1.0)
        return rs, nmurs

    # ---------- FF block ----------
    def _ff(self, s, z3, wk):
        nc = self.nc
        ones, idn = self.ones, self.idn
        rows, cpk = self.rows, self.cpk
        rs, nmurs = self._ln_stats(z3, wk, "pre")
        # h = (z3-mu)*rs, overwrite z3 tiles in place via scratch
        h_ = [wk.tile([128, D], F32, tag=f"B2_{tt}", name=f"h{tt}")
              for tt in range(NT)]
        for tt in range(NT):
            nc.scalar.activation(h_[tt][:], z3[tt][:], AF.Identity,
                                 scale=rs[:, tt:tt + 1], bias=nmurs[:, tt:tt + 1])
        hT = [wk.tile([128, N], F32, tag=f"A2_{dt}", name=f"hT{dt}")
              for dt in range(ND)]
        for dt in range(ND):
            for h in range(2):
                pT = self.bank(dt % 2)
                for q in range(4):
                    nc.tensor.transpose(pT[:, _sl(q)], h_[h * 4 + q][:, _sl(dt)],
                                        idn[:])
                nc.scalar.copy(hT[dt][:, _hh(h)], pT[:])
        znT = [wk.tile([128, N], F32, tag=f"A1_{dt}", name=f"znT{dt}")
               for dt in range(ND)]
        for dt in range(ND):
            nc.scalar.activation(znT[dt][:], hT[dt][:], AF.Identity,
                                 scale=cpk[:, dt:dt + 1], bias=cpk[:, 4 + dt:5 + dt])

        yT = [wk.tile([128, N], F32, tag=f"A2_{dt}", name=f"yT{dt}")
              for dt in range(ND)]
        for h in range(2):
            # split znT h-half into bf16 hi/lo (cols 0:512 hi, 512:1024 lo)
            znb = [wk.tile([128, 1024], BF16, tag=f"B3_{kt}", name=f"znb{kt}")
                   for kt in range(ND)]
            for kt in range(ND):
                nc.vector.tensor_copy(znb[kt][:, 0:512], znT[kt][:, _hh(h)])
                nc.vector.tensor_sub(znb[kt][:, 512:1024], znT[kt][:, _hh(h)],
                                     znb[kt][:, 0:512])
            pzf = [self.bank(dt) for dt in range(ND)]
            for m in range(NM):
                w1mh = wk.tile([128, ND * 128], BF16, tag=f"w1mh{m % 2}",
                               name="w1mh")
                w1ml = wk.tile([128, ND * 128], BF16, tag=f"w1ml{m % 2}",
                               name="w1ml")
                for kt in range(ND):
                    nc.sync.dma_start(w1mh[:, _sl(kt)], self.d_ffw1h[_sl(kt), _sl(m)])
                    nc.sync.dma_start(w1ml[:, _sl(kt)], self.d_ffw1l[_sl(kt), _sl(m)])
                ph = self.bank(4 + m % 2)
                for kt in range(ND):
                    nc.tensor.matmul(ph[:], w1mh[:, _sl(kt)], znb[kt][:, 0:512],
                                     start=(kt == 0), stop=False)
                    nc.tensor.matmul(ph[:], w1mh[:, _sl(kt)], znb[kt][:, 512:1024],
                                     start=False, stop=False)
                    nc.tensor.matmul(ph[:], w1ml[:, _sl(kt)], znb[kt][:, 0:512],
                                     start=False, stop=(kt == ND - 1))
                sig = wk.tile([128, 512], F32, tag=f"sig{m % 2}", name="sig")
                nc.scalar.activation(sig[:], ph[:], AF.Sigmoid,
                                     bias=cpk[:, 8 + m:9 + m])
                w2m = wk.tile([128, 512], F32, tag=f"w2m{m % 2}", name="w2m")
                nc.sync.dma_start(w2m[:], self.d_ffw2[_sl(m), :])
                for dt in range(ND):
                    nc.tensor.matmul(pzf[dt][:], w2m[:, _sl(dt)], sig[:],
                                     start=(m == 0), stop=(m == NM - 1))
            for dt in range(ND):
                nc.vector.scalar_tensor_tensor(yT[dt][:, _hh(h)], pzf[dt][:],
                                               cpk[:, 24 + dt:25 + dt],
                                               znT[dt][:, _hh(h)],
                                               op0=ALU.add, op1=ALU.add)

        y = [wk.tile([128, D], F32, tag=f"B4_{tt}", name=f"y{tt}")
             for tt in range(NT)]
        for tt in range(NT):
            pT = self.bank(6)
            for dt in range(ND):
                nc.tensor.transpose(pT[:, _sl(dt)], yT[dt][:, _sl(tt)], idn[:])
            nc.scalar.copy(y[tt][:], pT[:])

        rs2, nmurs2 = self._ln_stats(y, wk, "post")
        gb = wk.tile([128, D], F32, tag="gpb", name="gpb")
        pb = self.bank(0)
        nc.tensor.matmul(pb[:], ones[32:33, 0:128], rows[32:33, 512:1024],
                         start=True, stop=True)
        nc.scalar.copy(gb[:], pb[:])
        bb = wk.tile([128, D], F32, tag="bpb", name="bpb")
        pb2 = self.bank(1)
        nc.tensor.matmul(pb2[:], ones[32:33, 0:128], rows[32:33, 1024:1536],
                         start=True, stop=True)
        nc.scalar.copy(bb[:], pb2[:])
        z4 = [wk.tile([128, D], F32, tag=f"B1_{tt}", name=f"z4_{tt}")
              for tt in range(NT)]
        scr2 = wk.tile([128, D], F32, tag="lnscr", name="scr2")
        for tt in range(NT):
            nc.scalar.activation(scr2[:], y[tt][:], AF.Identity,
                                 scale=rs2[:, tt:tt + 1], bias=nmurs2[:, tt:tt + 1])
            nc.vector.tensor_mul(z4[tt][:], scr2[:], gb[:])
            nc.vector.tensor_add(z4[tt][:], z4[tt][:], bb[:])
        return z4

    # ---------- output head ----------
    def _output(self, s, wk):
        nc = self.nc
        ones = self.ones
        po = self.bank(7)
        for kt in range(ND):
            nc.tensor.matmul(po[0:TF, 0:HOR], self.outwt[:, kt * TF:(kt + 1) * TF],
                             self.aggt[:, (s * ND + kt) * HOR:(s * ND + kt + 1) * HOR],
                             start=(kt == 0), stop=False)
        nc.tensor.matmul(po[0:TF, 0:HOR], self.rows[64:65, 0:TF],
                         ones[64:65, 0:HOR], start=False, stop=True)
        xfin = wk.tile([TF, N], F32, tag="xts", name="xfin")
        nc.sync.dma_start(xfin[:], self.xtmid[s, :, :])
        oT = wk.tile([TF, HOR], F32, tag="oT", name="oT")
        nc.vector.tensor_scalar_add(oT[:], po[0:TF, 0:HOR], xfin[:, N - 1:N])
        nc.sync.dma_start(self.d_out[s * TF:(s + 1) * TF, :], oT[:])


def _get_nc():
    if "nc" not in _CACHE:
        _CACHE["nc"] = K().build()
    return _CACHE["nc"]


def _common_maps(inputs, w2d, dft, ib, e8):
    return dict(
        w2d=w2d,
        convb=np.asarray(inputs["conv_b"], np.float32).reshape(1, D),
        dft=dft, ib=ib,
        idn=np.eye(128, dtype=np.float32),
        e8=e8,
        win=np.asarray(inputs["mhesa_win"], np.float32),
        wout=np.asarray(inputs["mhesa_wout"], np.float32),
        binr=np.asarray(inputs["mhesa_bin"], np.float32),
        boutr=np.asarray(inputs["mhesa_bout"], np.float32).reshape(L, 1, D),
        initf=np.asarray(inputs["mhesa_init"], np.float32).reshape(L, D),
        alpha8=np.asarray(inputs["mhesa_alpha"], np.float32).reshape(L, HEADS, 1),
        ffw1=np.asarray(inputs["ff_w1"], np.float32),
        ffw1h=_split_hi(np.asarray(inputs["ff_w1"], np.float32)),
        ffw1l=_split_lo(np.asarray(inputs["ff_w1"], np.float32)),
        ffb1=np.asarray(inputs["ff_b1"], np.float32).reshape(1, FD),
        ffw2=np.asarray(inputs["ff_w2"], np.float32),
        ffb2=np.asarray(inputs["ff_b2"], np.float32).reshape(1, D),
        gprec=np.asarray(inputs["ff_pre_g"], np.float32).reshape(D, 1),
        bprec=np.asarray(inputs["ff_pre_b"], np.float32).reshape(D, 1),
        gpostr=np.asarray(inputs["ff_post_g"], np.float32).reshape(1, D),
        bpostr=np.asarray(inputs["ff_post_b"], np.float32).reshape(1, D),
        lvwg=np.asarray(inputs["level_wg"], np.float32),
        lvwp=np.asarray(inputs["level_wp"], np.float32),
        lvbg=np.asarray(inputs["level_bg"], np.float32).reshape(L, 1, TF),
        lvbp=np.asarray(inputs["level_bp"], np.float32).reshape(L, 1, TF),
        lvalpha=np.asarray(inputs["level_alpha"], np.float32).reshape(L, 1, 1),
        damp8=np.asarray(inputs["dampen_factor"], np.float32).reshape(HEADS, 1),
        outw=np.asarray(inputs["out_w"], np.float32),
        outbr=np.asarray(inputs["out_b"], np.float32).reshape(1, TF),
    )


def _kernel_impl(inputs, runner):
    x = np.asarray(inputs["x"], np.float32)
    assert (x.shape[0], x.shape[1], x.shape[2]) == (32, N, TF)
    assert int(inputs["forecast_horizon"]) == HOR
    dft, ib = _dft_consts()
    conv_w = np.asarray(inputs["conv_w"], np.float32)
    w2d = _build_w2d(conv_w)
    e8 = np.repeat(np.eye(HEADS, dtype=np.float32), DH, axis=1)
    nc = _get_nc()
    common = _common_maps(inputs, w2d, dft, ib, e8)
    in_maps = []
    for c in range(NCORES):
        xs = x[c * S:(c + 1) * S]
        xT = xs.transpose(0, 2, 1).reshape(S * TF, N).copy()
        in_maps.append(dict(common, xT=xT))
    res = runner(nc, in_maps)
    out = np.zeros((x.shape[0], HOR, TF), np.float32)
    for c in range(NCORES):
        oT = res.results[c]["outT"].reshape(S, TF, HOR)
        out[c * S:(c + 1) * S] = oT.transpose(0, 2, 1)
    return out, res


def kernel(**inputs):
    out, _ = _kernel_impl(
        inputs,
        lambda nc, im: run_bass_kernel_spmd(nc, im, list(range(NCORES))))
    return out


def kernel_traced(**inputs):
    """Like kernel() but with NTFF profiling; returns (out, BassKernelResults)."""
    return _kernel_impl(
        inputs,
        lambda nc, im: run_bass_kernel_spmd(nc, im, list(range(NCORES)),
                                            trace=True))
